# revision 1
# baseline (speedup 1.0000x reference)
"""Trainium2 Bass kernel for nn_DinoGazeSpade (segment_reduce + SPADE stack).

Strategy (8 NeuronCores, SPMD single program):
  - Two groups of 4 cores; group = batch index b (0..1), rank g = core % 4.
  - Painted-map + bilinear resize is reformulated as segment matrices:
        sem_rs[c,p,q] = sum_s avg[s,c] * M[s,p,q],
        M[s] = R @ onehot_s @ C^T   (R, C: 32x448 separable resize matrices)
    Each core builds M / avg for its 16 segments, computes a partial sem_rs,
    then an in-group AllReduce (C1) combines them (plus LayerNorm0 stats).
  - The three shared 3x3 convs (w_sh*) are split one-layer-per-core (ranks
    0..2); post-relu results are AllGathered (C1.5).
  - gamma/beta convs of layer 0 are split by output channel (384 per core);
    the pointwise c0 conv partials are AllReduced (C2).  Layers 1-2 are tiny
    and replicated.
  - conv3x3 = 9 shifted matmuls over a zero-padded [C,34,34] SBUF image.

The host side packs per-core shards / weight transposes (layout only) and
reassembles the [2,1,32,32] output from cores 0 and 4.
"""

import numpy as np

from concourse import bass, tile, mybir
from concourse.bass_utils import run_bass_kernel_spmd

F32 = mybir.dt.float32
BF16 = mybir.dt.bfloat16
BF16_NP = mybir.dt.np(BF16)
AOT = mybir.ActivationFunctionType
ALU = mybir.AluOpType

# Problem dims
B, CM, CS, HP, WP, HI, WI, HM, NSEG = 2, 1536, 384, 32, 32, 448, 448, 128, 64
G = 4              # cores per batch group
SEGC = NSEG // G   # segments per core = 16
COC = CM // G      # gamma/beta out-channel chunk per core = 384
NPIX = HP * WP     # 1024
EPS = 1e-12
LN0_N = float(CM * NPIX)
LN1_N = float(8 * NPIX)
LN2_N = float(16 * NPIX)

_NC_CACHE = {}


def _resize_matrix(n_in, n_out):
    """Row matrix of jax.image.resize(..., 'bilinear') for downsampling
    (antialiased triangle kernel, normalized rows). Verified vs jax."""
    scale = n_out / n_in
    p = np.arange(n_out, dtype=np.float64)[:, None]
    i = np.arange(n_in, dtype=np.float64)[None, :]
    center = (p + 0.5) / scale - 0.5
    w = np.maximum(0.0, 1.0 - np.abs(i - center) * scale)
    w = w / w.sum(axis=1, keepdims=True)
    return w.astype(np.float32)


def _split_sync_waits(nc, max_waits=1):
    """walrus in this container encodes at most one sync-wait per
    instruction; hoist extras onto preceding same-engine NoOps."""
    n = 0
    for fn in nc.m.functions:
        for blk in fn.blocks:
            new_insts = []
            for inst in blk.instructions:
                si = getattr(inst, "sync_info", None)
                if si is not None and si.on_wait and len(si.on_wait) > max_waits:
                    waits = list(si.on_wait)
                    head, rest = waits[:-max_waits], waits[-max_waits:]
                    for i in range(0, len(head), max_waits):
                        new_insts.append(mybir.InstNoOp(
                            name=f"I-ws-{nc.next_id()}", engine=inst.engine,
                            ins=[], outs=[],
                            sync_info=mybir.SyncInfo(
                                on_wait=list(head[i:i + max_waits]), on_update=[]),
                        ))
                    si.on_wait = rest
                    n += 1
                new_insts.append(inst)
            blk.instructions = new_insts
    return n


def _conv_windows(pad_ap, rows, cols, dy, dx, row0=0):
    """AP view [P, rows, cols] of a padded [P, 34, 34] image at tap (dy,dx)."""
    return pad_ap[:, row0 + dy:row0 + dy + rows, dx:dx + cols]


def _build_nc():
    nc = bass.Bass()

    def inp(name, shape, dtype):
        return nc.declare_dram_parameter(name, list(shape), dtype, isOutput=False)

    # --- inputs (per-core packed shards; see _pack_inputs) ---
    segbf = inp("segbf", [112, 4, 448], BF16)
    segval = inp("segval", [128, SEGC], F32)
    rt = inp("rt", [112, 4, 32], BF16)
    ct = inp("ct", [112, 4, 32], BF16)
    ident = inp("ident", [32, 32], BF16)
    segsm = inp("segsm", [128, 8], BF16)
    fsemT = inp("fsemT", [128, 8, 385], BF16)
    xq = inp("xq", [128, 3, NPIX], F32)
    wshl = inp("wshl", [128, 9, 3, 128], BF16)    # my layer's w_sh: [ci_p, tap, cic, co]
    bshl = inp("bshl", [128, 1], F32)             # my layer's b_sh
    wg = inp("wg", [128, 9, COC], BF16)           # w_g0 chunk:  [ci, tap, co_local]
    wbe = inp("wbe", [128, 9, COC], BF16)
    opg0 = inp("opg0", [128, 3], F32)      # 1 + b_g0 chunk, [ci_p, m]
    bbe0a = inp("bbe0a", [128, 3], F32)    # b_be0 chunk, [ci_p, m]
    wc0t = inp("wc0t", [128, 3, 8], BF16)
    wg1t = inp("wg1t", [128, 9, 8], BF16)
    wbe1t = inp("wbe1t", [128, 9, 8], BF16)
    opg1 = inp("opg1", [8, 1], F32)
    bbe1a = inp("bbe1a", [8, 1], F32)
    wg2t = inp("wg2t", [128, 9, 16], BF16)
    wbe2t = inp("wbe2t", [128, 9, 16], BF16)
    opg2 = inp("opg2", [16, 1], F32)
    bbe2a = inp("bbe2a", [16, 1], F32)
    wc1t = inp("wc1t", [8, 16], BF16)
    wc2t = inp("wc2t", [16, 1], BF16)
    b0 = inp("b0", [8, 1], F32)
    b1 = inp("b1", [16, 1], F32)
    b2 = inp("b2", [1, 1], F32)
    ones128f = inp("ones128f", [128, 1], F32)  # col of ones (reduce lhsT)

    out_t = nc.declare_dram_parameter("out", [1, NPIX], F32, isOutput=True)

    with tile.TileContext(nc) as tc:
        with (
            tc.tile_pool(name="const", bufs=1) as cpool,
            tc.tile_pool(name="work", bufs=1) as wpool,
            tc.tile_pool(name="seg", bufs=3) as segpool,
            tc.tile_pool(name="dram", bufs=1, space="DRAM") as dpool,
        ):
            # ---------- load constants / inputs into SBUF ----------
            def load(pool, ap, dtype=None, name=None):
                t = pool.tile(list(ap.shape), dtype or ap.dtype, tag=name)
                nc.sync.dma_start(out=t[:], in_=ap[:])
                return t

            seg_sb = load(cpool, segbf, name="seg_sb")
            segval_sb = load(cpool, segval, name="segval_sb")
            rt_sb = load(cpool, rt, name="rt_sb")
            ct_sb = load(cpool, ct, name="ct_sb")
            ident_sb = load(cpool, ident, name="ident_sb")
            segsm_sb = load(cpool, segsm, name="segsm_sb")
            fsemT_sb = load(cpool, fsemT, name="fsemT_sb")
            x_sb = load(cpool, xq, name="x_sb")
            wshl_sb = load(cpool, wshl, name="wshl_sb")
            bshl_sb = load(cpool, bshl, name="bshl_sb")
            wg_sb = load(cpool, wg, name="wg_sb")
            wbe_sb = load(cpool, wbe, name="wbe_sb")
            opg0_sb = load(cpool, opg0, name="opg0_sb")
            bbe0a_sb = load(cpool, bbe0a, name="bbe0a_sb")
            wc0t_sb = load(cpool, wc0t, name="wc0t_sb")
            wg1t_sb = load(cpool, wg1t, name="wg1t_sb")
            wbe1t_sb = load(cpool, wbe1t, name="wbe1t_sb")
            opg1_sb = load(cpool, opg1, name="opg1_sb")
            bbe1a_sb = load(cpool, bbe1a, name="bbe1a_sb")
            wg2t_sb = load(cpool, wg2t, name="wg2t_sb")
            wbe2t_sb = load(cpool, wbe2t, name="wbe2t_sb")
            opg2_sb = load(cpool, opg2, name="opg2_sb")
            bbe2a_sb = load(cpool, bbe2a, name="bbe2a_sb")
            wc1t_sb = load(cpool, wc1t, name="wc1t_sb")
            wc2t_sb = load(cpool, wc2t, name="wc2t_sb")
            b0_sb = load(cpool, b0, name="b0_sb")
            b1_sb = load(cpool, b1, name="b1_sb")
            b2_sb = load(cpool, b2, name="b2_sb")
            ones128f_sb = load(cpool, ones128f, name="ones128f_sb")

            # DRAM scratch
            m_dram = dpool.tile([32, SEGC, 32], F32)
            crs_in = dpool.tile([CS + 1, NPIX], BF16)
            crs_out = dpool.tile([CS + 1, NPIX], BF16)
            chs_in = dpool.tile([128, NPIX], BF16)
            chs_out = dpool.tile([G, 128, NPIX], BF16)
            cc0_in = dpool.tile([8, NPIX], F32)
            cc0_out = dpool.tile([8, NPIX], F32)

            # ---------- Phase A1: LayerNorm0 partial stats from x chunk ----------
            xsum = wpool.tile([128, 1], F32, tag="xsum")
            nc.vector.tensor_reduce(xsum[:], x_sb[:], mybir.AxisListType.XY, ALU.add)
            xsq_scratch = wpool.tile([128, 3, NPIX], BF16, tag="sq_scratch")
            xsumsq = wpool.tile([128, 1], F32, tag="xsumsq")
            nc.scalar.activation(xsq_scratch[:], x_sb[:], AOT.Square,
                                 accum_out=xsumsq[:])
            stats2 = wpool.tile([128, 2], F32, tag="stats2")
            nc.vector.tensor_copy(stats2[:, 0:1], xsum[:])
            nc.vector.tensor_copy(stats2[:, 1:2], xsumsq[:])

            psA_ctx = tc.tile_pool(name="psA", bufs=1, space="PSUM")
            psA = psA_ctx.__enter__()

            # cross-partition reduce of LN0 partial stats via ones-matmul
            stats1_ps = psA.tile([1, 2], F32, tag="stx")
            nc.tensor.matmul(stats1_ps[:], lhsT=ones128f_sb[:], rhs=stats2[:],
                             start=True, stop=True)
            stats1_sb = wpool.tile([1, 2], F32, tag="stats1_sb")
            nc.scalar.activation(stats1_sb[:], stats1_ps[:], AOT.Copy)

            # ---------- Phase A2: segment averages for my 16 segments ----------
            ohsm = wpool.tile([128, SEGC, 8], BF16, tag="ohsm")
            for s in range(SEGC):
                nc.vector.tensor_scalar(ohsm[:, s, :], segsm_sb[:],
                                        segval_sb[:, s:s + 1], None, ALU.is_equal)
            sums_ps = psA.tile([SEGC, 385], F32, tag="sums")
            for c in range(8):
                nc.tensor.matmul(sums_ps[:], lhsT=ohsm[:, :, c],
                                 rhs=fsemT_sb[:, c, :],
                                 start=(c == 0), stop=(c == 7))
            sums_sb = wpool.tile([SEGC, 385], F32, tag="sums_sb")
            nc.scalar.activation(sums_sb[:], sums_ps[:], AOT.Copy)
            cnt_safe = wpool.tile([SEGC, 1], F32, tag="cnt_safe")
            nc.vector.tensor_scalar(cnt_safe[:], sums_sb[:, 384:385], 1.0, None,
                                    ALU.max)
            rec = wpool.tile([SEGC, 1], F32, tag="rec")
            nc.vector.reciprocal(rec[:], cnt_safe[:])
            mask = wpool.tile([SEGC, 1], F32, tag="mask")
            nc.vector.tensor_scalar(mask[:], sums_sb[:, 384:385], 0.5, None,
                                    ALU.is_gt)
            recm = wpool.tile([SEGC, 1], F32, tag="recm")
            nc.vector.tensor_mul(recm[:], rec[:], mask[:])
            avg_bf = wpool.tile([SEGC, CS], BF16, tag="avg_bf")
            nc.vector.tensor_scalar(avg_bf[:], sums_sb[:, 0:384], recm[:], None,
                                    ALU.mult)

            # ---------- Phase A3: M matrices for my 16 segments ----------
            mall_ps = psA.tile([32, SEGC * 32], F32, tag="mall")
            for s in range(SEGC):
                oh = segpool.tile([112, 4, 448], BF16, tag="oh")
                nc.vector.tensor_scalar(oh[:], seg_sb[:],
                                        segval_sb[0:112, s:s + 1], None,
                                        ALU.is_equal)
                a_ps = psA.tile([32, 448], F32, tag="aps", bufs=2)
                for c in range(4):
                    nc.tensor.matmul(a_ps[:], lhsT=rt_sb[:, c, :],
                                     rhs=oh[:, c, :],
                                     start=(c == 0), stop=(c == 3))
                a_sb = segpool.tile([32, 448], BF16, tag="asb")
                nc.scalar.activation(a_sb[:], a_ps[:], AOT.Copy)
                at_ps = psA.tile([112, 4, 32], BF16, tag="atps", bufs=2)
                for c in range(4):
                    nc.tensor.transpose(at_ps[:, c, :],
                                        a_sb[:, c * 112:(c + 1) * 112],
                                        ident_sb[:])
                at_sb = segpool.tile([112, 4, 32], BF16, tag="atsb")
                nc.scalar.activation(at_sb[:], at_ps[:], AOT.Copy)
                for c in range(4):
                    nc.tensor.matmul(mall_ps[:, s * 32:(s + 1) * 32],
                                     lhsT=at_sb[:, c, :], rhs=ct_sb[:, c, :],
                                     start=(c == 0), stop=(c == 3))

            # M [32(p), 16(s), 32(q)] -> DRAM -> back as [16(s), 32(p)*32(q)]
            mall_sb = wpool.tile([32, SEGC * 32], F32, tag="mall_sb")
            nc.scalar.activation(mall_sb[:], mall_ps[:], AOT.Copy)
            nc.sync.dma_start(out=m_dram[:], in_=mall_sb[:])
            mt_f32 = wpool.tile([SEGC, NPIX], F32, tag="mt_f32")
            nc.sync.dma_start(out=mt_f32[:],
                              in_=m_dram[:].rearrange("p s q -> s p q"))
            mt_bf = wpool.tile([SEGC, NPIX], BF16, tag="mt_bf")
            nc.vector.tensor_copy(mt_bf[:], mt_f32[:])

            psA_ctx.__exit__(None, None, None)
            psB_ctx = tc.tile_pool(name="psB", bufs=1, space="PSUM")
            psB = psB_ctx.__enter__()
            # ---------- Phase A4: partial sem_rs + C1 AllReduce ----------
            for k in range(3):
                srs_sb = wpool.tile([128, NPIX], BF16, tag="srs_sb", bufs=2)
                for h in range(2):
                    srs_ps = psB.tile([128, 512], F32, tag="big0", bufs=4)
                    nc.tensor.matmul(srs_ps[:],
                                     lhsT=avg_bf[:, k * 128:(k + 1) * 128],
                                     rhs=mt_bf[:, h * 512:(h + 1) * 512],
                                     start=True, stop=True)
                    nc.scalar.activation(srs_sb[:, h * 512:(h + 1) * 512],
                                         srs_ps[:], AOT.Copy)
                nc.sync.dma_start(out=crs_in[k * 128:(k + 1) * 128, :],
                                  in_=srs_sb[:])
            stats1_bf = wpool.tile([1, 2], BF16, tag="stats1_bf")
            nc.vector.tensor_copy(stats1_bf[:], stats1_sb[:])
            nc.sync.dma_start(out=crs_in[CS:CS + 1, 0:2], in_=stats1_bf[:])

            nc.gpsimd.collective_compute(
                "AllReduce", ALU.add,
                replica_groups=[[0, 1, 2, 3], [4, 5, 6, 7]],
                ins=[crs_in[:]], outs=[crs_out[:]],
            )

            # ---------- Phase B0: LN0 scalars ----------
            st0_bc = wpool.tile([128, 2], BF16, tag="st0_bc")
            nc.sync.dma_start(
                out=st0_bc[:],
                in_=crs_out[CS:CS + 1, 0:2].partition_broadcast(128))

            def ln_from_bc(st_bc, n_elems, nparts, tag):
                """st_bc [nparts,2]=(sum,sumsq) replicated -> mu,istd [nparts,1].
                istd = exp(-0.5*ln(var+eps)) keeps ACT in the exp/ln set."""
                ms = wpool.tile([nparts, 2], F32, tag=tag + "_ms")
                nc.vector.tensor_scalar(ms[:], st_bc[:], 1.0 / n_elems, None,
                                        ALU.mult)
                musq = wpool.tile([nparts, 1], F32, tag=tag + "_musq")
                nc.vector.tensor_mul(musq[:], ms[:, 0:1], ms[:, 0:1])
                var = wpool.tile([nparts, 1], F32, tag=tag + "_var")
                nc.vector.tensor_sub(var[:], ms[:, 1:2], musq[:])
                vare = wpool.tile([nparts, 1], F32, tag=tag + "_vare")
                nc.vector.tensor_scalar(vare[:], var[:], EPS, None, ALU.add)
                lnv = wpool.tile([nparts, 1], F32, tag=tag + "_lnv")
                nc.scalar.activation(lnv[:], vare[:], AOT.Ln)
                istd = wpool.tile([nparts, 1], F32, tag=tag + "_istd")
                nc.scalar.activation(istd[:], lnv[:], AOT.Exp, scale=-0.5)
                return ms[:, 0:1], istd

            mu0_bc, istd0_bc = ln_from_bc(st0_bc, LN0_N, 128, "ln0")

            # ---------- Phase B1: padded sem_rs in SBUF (bf16) ----------
            semrs_pad = []
            for k in range(3):
                sp = wpool.tile([128, 34, 34], BF16, tag=f"semrs_pad{k}")
                nc.vector.memset(sp[:], 0.0)
                nc.sync.dma_start(
                    out=sp[:, 1:33, 1:33],
                    in_=crs_out[k * 128:(k + 1) * 128, :].rearrange(
                        "c (h w) -> c h w", h=32))
                semrs_pad.append(sp)

            # ---------- Phase B2: my shared conv layer (rank g = layer) ----------
            hsh_mine = wpool.tile([128, NPIX], BF16, tag="hsh_mine")
            for h in range(2):
                hsh_ps = psB.tile([128, 512], F32, tag="big0", bufs=4)
                first = True
                for cic in range(3):
                    for t in range(9):
                        dy, dx = t // 3, t % 3
                        nc.tensor.matmul(
                            hsh_ps[:].rearrange("c (r w) -> c r w", r=16),
                            lhsT=wshl_sb[:, t, cic, :],
                            rhs=_conv_windows(semrs_pad[cic][:], 16, 32, dy, dx,
                                              row0=h * 16),
                            start=first, stop=(cic == 2 and t == 8))
                        first = False
                nc.scalar.activation(hsh_mine[:, h * 512:(h + 1) * 512],
                                     hsh_ps[:], AOT.Relu, bias=bshl_sb[:])
            nc.sync.dma_start(out=chs_in[:], in_=hsh_mine[:])

            nc.gpsimd.collective_compute(
                "AllGather", ALU.bypass,
                replica_groups=[[0, 1, 2, 3], [4, 5, 6, 7]],
                ins=[chs_in[:]], outs=[chs_out[:]],
            )

            # ---------- Phase B4: padded h_sh images ----------
            hsh_pad = []
            for L in range(3):
                hp = wpool.tile([128, 34, 34], BF16, tag=f"hsh_pad{L}")
                nc.vector.memset(hp[:], 0.0)
                nc.sync.dma_start(
                    out=hp[:, 1:33, 1:33],
                    in_=chs_out[L][:].rearrange("c (h w) -> c h w", h=32))
                hsh_pad.append(hp)

            # ---------- Phase B5/B6: layer-0 gamma/beta convs + SPADE + c0 ----------
            c0_ps = [psB.tile([8, 512], F32, tag="acc", bufs=2,
                              name=f"c0_ps{h}") for h in range(2)]
            for m in range(3):
                xn_m = wpool.tile([128, NPIX], BF16, tag="xn", bufs=2)
                nc.vector.tensor_scalar(xn_m[:], x_sb[:, m, :], mu0_bc[:],
                                        istd0_bc[:], ALU.subtract, ALU.mult)
                for h in range(2):
                    gb_ps = {}
                    for name, w_sb in (("g", wg_sb), ("be", wbe_sb)):
                        ps = psB.tile([128, 512], F32,
                                      tag=("big0" if name == "g" else "big1"),
                                      bufs=(4 if name == "g" else 2))
                        for t in range(9):
                            dy, dx = t // 3, t % 3
                            nc.tensor.matmul(
                                ps[:].rearrange("c (r w) -> c r w", r=16),
                                lhsT=w_sb[:, t, m * 128:(m + 1) * 128],
                                rhs=_conv_windows(hsh_pad[0][:], 16, 32, dy, dx,
                                                  row0=h * 16),
                                start=(t == 0), stop=(t == 8))
                        gb_ps[name] = ps
                    # spade: out = xn*(1+gamma+bg) + (beta+bbe)
                    hs = slice(h * 512, (h + 1) * 512)
                    u = wpool.tile([128, 512], BF16, tag="spade_u", bufs=2)
                    nc.vector.scalar_tensor_tensor(u[:], gb_ps["g"][:],
                                                   opg0_sb[:, m:m + 1],
                                                   xn_m[:, hs],
                                                   ALU.add, ALU.mult)
                    sp0 = wpool.tile([128, 512], BF16, tag="spade_o", bufs=2)
                    nc.vector.scalar_tensor_tensor(sp0[:], gb_ps["be"][:],
                                                   bbe0a_sb[:, m:m + 1], u[:],
                                                   ALU.add, ALU.add)
                    nc.tensor.matmul(c0_ps[h][:], lhsT=wc0t_sb[:, m, :],
                                     rhs=sp0[:], start=(m == 0), stop=(m == 2))

            c0p_sb = wpool.tile([8, NPIX], F32, tag="f32buf")
            for h in range(2):
                nc.scalar.activation(c0p_sb[:, h * 512:(h + 1) * 512],
                                     c0_ps[h][:], AOT.Copy)
            nc.sync.dma_start(out=cc0_in[:], in_=c0p_sb[:])
            nc.gpsimd.collective_compute(
                "AllReduce", ALU.add,
                replica_groups=[[0, 1, 2, 3], [4, 5, 6, 7]],
                ins=[cc0_in[:]], outs=[cc0_out[:]],
            )


            def softplus_to(dst, src_ap, bias_sb, nparts, tag):
                """dst = softplus(src + bias) = max(t,0) + log1p(exp(-|t|))."""
                t = wpool.tile([nparts, NPIX], F32, tag="spx_t")
                nc.vector.tensor_scalar(t[:], src_ap, bias_sb[:], None, ALU.add)
                a = wpool.tile([nparts, NPIX], F32, tag="spx_a")
                nc.scalar.activation(a[:], t[:], AOT.Abs)
                e = wpool.tile([nparts, NPIX], F32, tag="spx_e")
                nc.scalar.activation(e[:], a[:], AOT.Exp, scale=-1.0)
                l = wpool.tile([nparts, NPIX], F32, tag="spx_l")
                nc.scalar.activation(l[:], e[:], AOT.Ln, bias=1.0)
                r = wpool.tile([nparts, NPIX], F32, tag="spx_r")
                nc.vector.tensor_scalar(r[:], t[:], 0.0, None, ALU.max)
                nc.vector.tensor_add(dst[:], r[:], l[:])

            # ---------- Phase B7: h1 + LN1 ----------
            c0_sb = wpool.tile([8, NPIX], F32, tag="f32buf")
            nc.sync.dma_start(out=c0_sb[:], in_=cc0_out[:])
            h1_f32 = wpool.tile([8, NPIX], F32, tag="hbuf")
            softplus_to(h1_f32, c0_sb[:], b0_sb, 8, "sp1h")

            def ln_small(h_f32, nparts, n_elems, tag):
                hsum = wpool.tile([nparts, 1], F32, tag=tag + "_hsum")
                nc.vector.tensor_reduce(hsum[:], h_f32[:], mybir.AxisListType.X,
                                        ALU.add)
                hsq = wpool.tile([nparts, NPIX], BF16, tag="sq_scratch")
                hsumsq = wpool.tile([nparts, 1], F32, tag=tag + "_hsumsq")
                nc.scalar.activation(hsq[:], h_f32[:], AOT.Square,
                                     accum_out=hsumsq[:])
                st2 = wpool.tile([nparts, 2], F32, tag=tag + "_st2")
                nc.vector.tensor_copy(st2[:, 0:1], hsum[:])
                nc.vector.tensor_copy(st2[:, 1:2], hsumsq[:])
                st1_ps = psB.tile([1, 2], F32, tag="acc", bufs=2)
                nc.tensor.matmul(st1_ps[:], lhsT=ones128f_sb[0:nparts, :],
                                 rhs=st2[:], start=True, stop=True)
                st1_sb = wpool.tile([1, 2], F32, tag=tag + "_st1")
                nc.scalar.activation(st1_sb[:], st1_ps[:], AOT.Copy)
                sdram = dpool.tile([1, 2], F32, name=tag + "_sdram")
                nc.sync.dma_start(out=sdram[:], in_=st1_sb[:])
                st_bc = wpool.tile([nparts, 2], F32, tag=tag + "_stbc")
                nc.sync.dma_start(out=st_bc[:],
                                  in_=sdram[:].partition_broadcast(nparts))
                return ln_from_bc(st_bc, n_elems, nparts, tag)

            mu1_bc, istd1_bc = ln_small(h1_f32, 8, LN1_N, "ln1")
            xn1 = wpool.tile([8, NPIX], BF16, tag="xn1")
            nc.vector.tensor_scalar(xn1[:], h1_f32[:], mu1_bc[:], istd1_bc[:],
                                    ALU.subtract, ALU.mult)

            # ---------- Phase B8: layers 1 and 2 (replicated) ----------
            def spade_small(xn_bf, nco, wgt_sb, opg_sb, wbet_sb, bbea_sb, tag):
                sp_ = wpool.tile([nco, NPIX], BF16, tag=f"{tag}_sp")
                pad_img = hsh_pad[1 if tag == "l1" else 2]
                for h in range(2):
                    ps = {}
                    for name, w_sb in (("g", wgt_sb), ("be", wbet_sb)):
                        p = psB.tile([nco, 512], F32,
                                     tag=("big0" if name == "g" else "big1"),
                                     bufs=(4 if name == "g" else 2))
                        for t in range(9):
                            dy, dx = t // 3, t % 3
                            nc.tensor.matmul(
                                p[:].rearrange("c (r w) -> c r w", r=16),
                                lhsT=w_sb[:, t, :],
                                rhs=_conv_windows(pad_img[:], 16, 32, dy, dx,
                                                  row0=h * 16),
                                start=(t == 0), stop=(t == 8))
                        ps[name] = p
                    hs = slice(h * 512, (h + 1) * 512)
                    u_ = wpool.tile([nco, 512], BF16, tag=f"{tag}_u")
                    nc.vector.scalar_tensor_tensor(u_[:], ps["g"][:], opg_sb[:],
                                                   xn_bf[:, hs],
                                                   ALU.add, ALU.mult)
                    nc.vector.scalar_tensor_tensor(sp_[:, hs], ps["be"][:],
                                                   bbea_sb[:], u_[:],
                                                   ALU.add, ALU.add)
                return sp_

            sp1 = spade_small(xn1, 8, wg1t_sb, opg1_sb, wbe1t_sb, bbe1a_sb, "l1")
            c1_sb = wpool.tile([16, NPIX], F32, tag="f32buf")
            for h in range(2):
                c1_ps = psB.tile([16, 512], F32, tag="acc", bufs=2)
                nc.tensor.matmul(c1_ps[:], lhsT=wc1t_sb[:],
                                 rhs=sp1[:, h * 512:(h + 1) * 512],
                                 start=True, stop=True)
                nc.scalar.activation(c1_sb[:, h * 512:(h + 1) * 512], c1_ps[:],
                                     AOT.Copy)
            h2_f32 = wpool.tile([16, NPIX], F32, tag="hbuf")
            softplus_to(h2_f32, c1_sb[:], b1_sb, 16, "sp2h")

            mu2_bc, istd2_bc = ln_small(h2_f32, 16, LN2_N, "ln2")
            xn2 = wpool.tile([16, NPIX], BF16, tag="xn2")
            nc.vector.tensor_scalar(xn2[:], h2_f32[:], mu2_bc[:], istd2_bc[:],
                                    ALU.subtract, ALU.mult)

            sp2 = spade_small(xn2, 16, wg2t_sb, opg2_sb, wbe2t_sb, bbe2a_sb, "l2")
            c2_sb = wpool.tile([1, NPIX], F32, tag="f32buf")
            for h in range(2):
                c2_ps = psB.tile([1, 512], F32, tag="acc", bufs=2)
                nc.tensor.matmul(c2_ps[:], lhsT=wc2t_sb[:],
                                 rhs=sp2[:, h * 512:(h + 1) * 512],
                                 start=True, stop=True)
                nc.scalar.activation(c2_sb[:, h * 512:(h + 1) * 512], c2_ps[:],
                                     AOT.Copy)
            out_f32 = wpool.tile([1, NPIX], F32, tag="hbuf")
            softplus_to(out_f32, c2_sb[:], b2_sb, 1, "sp3h")
            nc.sync.dma_start(out=out_t[:], in_=out_f32[:])
            psB_ctx.__exit__(None, None, None)

    nc.finalize()
    _split_sync_waits(nc)
    return nc


def _pack_inputs(inputs):
    f32 = np.float32
    R = _resize_matrix(HI, HP)      # [32, 448]
    C = _resize_matrix(WI, WP)      # [32, 448]
    rt = np.zeros((112, 4, 32), f32)
    ctm = np.zeros((112, 4, 32), f32)
    for c in range(4):
        rt[:, c, :] = R[:, c * 112:(c + 1) * 112].T
        ctm[:, c, :] = C[:, c * 112:(c + 1) * 112].T
    ident = np.eye(32, dtype=f32)

    segmap = inputs["segmap"]            # [2, 448, 448] int32
    f_sem = inputs["f_semantic"]         # [2, 384, 32, 32]
    x_main = inputs["x_main"]            # [2, 1536, 32, 32]
    rows = (np.arange(HP) * HI) // HP

    def tap_t(w):  # [co, ci, 3, 3] -> [ci, 9, co]
        return np.ascontiguousarray(w.transpose(1, 2, 3, 0).reshape(
            w.shape[1], 9, w.shape[0]))

    wsh_all = [inputs["w_sh0"], inputs["w_sh1"], inputs["w_sh2"]]
    bsh_all = [inputs["b_sh0"], inputs["b_sh1"], inputs["b_sh2"]]

    maps = []
    for cid in range(8):
        b, g = cid // G, cid % G
        d = {}
        seg = segmap[b].astype(f32)
        d["segbf"] = seg.reshape(4, 112, WI).transpose(1, 0, 2).astype(BF16_NP)
        d["segval"] = np.broadcast_to(
            (np.arange(SEGC, dtype=f32) + SEGC * g)[None, :], (128, SEGC)
        ).astype(f32).copy()
        d["rt"] = rt.astype(BF16_NP)
        d["ct"] = ctm.astype(BF16_NP)
        d["ident"] = ident.astype(BF16_NP)
        seg_small = seg[rows[:, None], rows[None, :]].reshape(-1)   # [1024]
        d["segsm"] = seg_small.reshape(8, 128).T.astype(BF16_NP).copy()
        fT = f_sem[b].reshape(CS, NPIX).T                           # [1024, 384]
        fTe = np.concatenate([fT, np.ones((NPIX, 1), f32)], 1)      # [1024, 385]
        d["fsemT"] = fTe.reshape(8, 128, 385).transpose(1, 0, 2).astype(
            BF16_NP).copy()
        xc = x_main[b, g * COC:(g + 1) * COC].reshape(COC, NPIX)
        d["xq"] = np.ascontiguousarray(
            xc.reshape(3, 128, NPIX).transpose(1, 0, 2))
        # my shared-conv layer (ranks 0..2; rank 3 gets zeros)
        if g < 3:
            wsh = wsh_all[g]   # [128, 384, 3, 3]
            wshl = wsh.transpose(1, 2, 3, 0).reshape(CS, 9, HM)  # [ci,9,co]
            d["wshl"] = np.ascontiguousarray(
                wshl.reshape(3, 128, 9, HM).transpose(1, 2, 0, 3)).astype(BF16_NP)
            d["bshl"] = bsh_all[g].reshape(128, 1).astype(f32)
        else:
            d["wshl"] = np.zeros((128, 9, 3, 128), BF16_NP)
            d["bshl"] = np.zeros((128, 1), f32)
        cosl = slice(g * COC, (g + 1) * COC)
        d["wg"] = tap_t(inputs["w_g0"][cosl]).astype(BF16_NP)      # [128,9,384]
        d["wbe"] = tap_t(inputs["w_be0"][cosl]).astype(BF16_NP)
        d["opg0"] = np.ascontiguousarray(
            (1.0 + inputs["b_g0"][cosl]).reshape(3, 128).T).astype(f32)
        d["bbe0a"] = np.ascontiguousarray(
            inputs["b_be0"][cosl].reshape(3, 128).T).astype(f32)
        wc0 = inputs["w_c0"][:, :, 0, 0]                           # [8, 1536]
        d["wc0t"] = np.ascontiguousarray(
            wc0[:, cosl].T.reshape(3, 128, 8).transpose(1, 0, 2)).astype(BF16_NP)
        d["wg1t"] = tap_t(inputs["w_g1"]).astype(BF16_NP)          # [128,9,8]
        d["wbe1t"] = tap_t(inputs["w_be1"]).astype(BF16_NP)
        d["opg1"] = (1.0 + inputs["b_g1"]).reshape(8, 1).astype(f32)
        d["bbe1a"] = inputs["b_be1"].reshape(8, 1).astype(f32)
        d["wg2t"] = tap_t(inputs["w_g2"]).astype(BF16_NP)          # [128,9,16]
        d["wbe2t"] = tap_t(inputs["w_be2"]).astype(BF16_NP)
        d["opg2"] = (1.0 + inputs["b_g2"]).reshape(16, 1).astype(f32)
        d["bbe2a"] = inputs["b_be2"].reshape(16, 1).astype(f32)
        d["wc1t"] = inputs["w_c1"][:, :, 0, 0].T.astype(BF16_NP).copy()  # [8,16]
        d["wc2t"] = inputs["w_c2"][:, :, 0, 0].T.astype(BF16_NP).copy()  # [16,1]
        d["b0"] = inputs["bias0"].reshape(8, 1).astype(f32)
        d["b1"] = inputs["bias1"].reshape(16, 1).astype(f32)
        d["b2"] = inputs["bias2"].reshape(1, 1).astype(f32)
        d["ones128f"] = np.ones((128, 1), f32)
        maps.append(d)
    return maps


def kernel(**inputs):
    if "nc" not in _NC_CACHE:
        _NC_CACHE["nc"] = _build_nc()
    nc = _NC_CACHE["nc"]
    in_maps = _pack_inputs(inputs)
    res = run_bass_kernel_spmd(nc, in_maps, list(range(8)))
    out = np.zeros((B, 1, HP, WP), np.float32)
    out[0, 0] = res.results[0]["out"].reshape(HP, WP)
    out[1, 0] = res.results[4]["out"].reshape(HP, WP)
    return out


if __name__ == "__main__":
    nc = _build_nc()
    print("built OK; instructions:",
          sum(len(b.instructions) for f in nc.m.functions for b in f.blocks))



# revision 9
# speedup vs baseline: 1.2956x; 1.2956x over previous
"""Trainium2 Bass kernel for nn_DinoGazeSpade (segment_reduce + SPADE stack).

Strategy (8 NeuronCores, SPMD single program):
  - Two groups of 4 cores; group = batch index b (0..1), rank g = core % 4.
  - Painted-map + bilinear resize is reformulated as segment matrices:
        sem_rs[c,p,q] = sum_s avg[s,c] * M[s,p,q],
        M[s] = R @ onehot_s @ C^T   (R, C: 32x448 separable resize matrices)
    Each core builds avg + M for its 16 segments only, then ONE small
    in-group AllGather ships (avg ‖ M ‖ LN0 stats) [17,1408] bf16 (~48KB);
    every core then computes the FULL sem_rs with a cheap k=64 matmul set.
  - The three shared 3x3 convs (w_sh*) are fully replicated (no AllGather):
    sh0 right after sem_rs; sh1/sh2 are emitted after the c0 AllReduce
    trigger so their matmuls overlap the collective.
  - gamma/beta convs of layer 0 are split by output channel (384 per core);
    the pointwise c0 conv partials are AllReduced (C2).  Layers 1-2 are tiny
    and replicated.
  - conv3x3 = 9 shifted matmuls over a zero-padded [C,34,34] SBUF image.
  - LayerNorm scalars use a ones-matmul reduce+broadcast (no DRAM roundtrip)
    and Rsqrt; softplus is the native ACT Softplus.

The host side packs per-core shards / weight transposes (layout only) and
reassembles the [2,1,32,32] output from cores 0 and 4.
"""

import numpy as np

from concourse import bass, tile, mybir
from concourse.bass_utils import run_bass_kernel_spmd

F32 = mybir.dt.float32
BF16 = mybir.dt.bfloat16
BF16_NP = mybir.dt.np(BF16)
AOT = mybir.ActivationFunctionType
ALU = mybir.AluOpType

# Problem dims
B, CM, CS, HP, WP, HI, WI, HM, NSEG = 2, 1536, 384, 32, 32, 448, 448, 128, 64
G = 4              # cores per batch group
SEGC = NSEG // G   # segments per core = 16
COC = CM // G      # gamma/beta out-channel chunk per core = 384
NPIX = HP * WP     # 1024
EPS = 1e-12
LN0_N = float(CM * NPIX)
LN1_N = float(8 * NPIX)
LN2_N = float(16 * NPIX)
PAYW = CS + NPIX   # 1408 payload cols: avg | M

_NC_CACHE = {}


def _resize_matrix(n_in, n_out):
    """Row matrix of jax.image.resize(..., 'bilinear') for downsampling
    (antialiased triangle kernel, normalized rows). Verified vs jax."""
    scale = n_out / n_in
    p = np.arange(n_out, dtype=np.float64)[:, None]
    i = np.arange(n_in, dtype=np.float64)[None, :]
    center = (p + 0.5) / scale - 0.5
    w = np.maximum(0.0, 1.0 - np.abs(i - center) * scale)
    w = w / w.sum(axis=1, keepdims=True)
    return w.astype(np.float32)


def _split_sync_waits(nc, max_waits=1):
    """walrus in this container encodes at most one sync-wait per
    instruction; hoist extras onto preceding same-engine NoOps."""
    n = 0
    for fn in nc.m.functions:
        for blk in fn.blocks:
            new_insts = []
            for inst in blk.instructions:
                si = getattr(inst, "sync_info", None)
                if si is not None and si.on_wait and len(si.on_wait) > max_waits:
                    waits = list(si.on_wait)
                    head, rest = waits[:-max_waits], waits[-max_waits:]
                    for i in range(0, len(head), max_waits):
                        new_insts.append(mybir.InstNoOp(
                            name=f"I-ws-{nc.next_id()}", engine=inst.engine,
                            ins=[], outs=[],
                            sync_info=mybir.SyncInfo(
                                on_wait=list(head[i:i + max_waits]), on_update=[]),
                        ))
                    si.on_wait = rest
                    n += 1
                new_insts.append(inst)
            blk.instructions = new_insts
    return n


def _conv_windows(pad_ap, rows, cols, dy, dx, row0=0):
    """AP view [P, rows, cols] of a padded [P, 34, 34] image at tap (dy,dx)."""
    return pad_ap[:, row0 + dy:row0 + dy + rows, dx:dx + cols]


def _build_nc():
    nc = bass.Bass()

    def inp(name, shape, dtype):
        return nc.declare_dram_parameter(name, list(shape), dtype, isOutput=False)

    # --- inputs (per-core packed shards; see _pack_inputs) ---
    segbf = inp("segbf", [112, 4, 448], BF16)
    segval = inp("segval", [128, SEGC], F32)
    rt = inp("rt", [112, 4, 32], BF16)
    ct = inp("ct", [112, 4, 32], BF16)
    ident = inp("ident", [32, 32], BF16)
    segsm = inp("segsm", [128, 8], BF16)
    fsemT = inp("fsemT", [128, 8, 385], BF16)
    xq = inp("xq", [128, 3, NPIX], F32)
    wsh3 = inp("wsh3", [128, 3, 9, 3, 128], BF16)  # all sh: [ci_p, L, tap, cic, co]
    bsh3 = inp("bsh3", [128, 3], F32)              # all b_sh: [co, L]
    wg = inp("wg", [128, 9, COC], BF16)            # w_g0 chunk:  [ci, tap, co_local]
    wbe = inp("wbe", [128, 9, COC], BF16)
    opg0 = inp("opg0", [128, 3], F32)      # 1 + b_g0 chunk, [ci_p, m]
    bbe0a = inp("bbe0a", [128, 3], F32)    # b_be0 chunk, [ci_p, m]
    wc0t = inp("wc0t", [128, 3, 8], BF16)
    wg1t = inp("wg1t", [128, 9, 8], BF16)
    wbe1t = inp("wbe1t", [128, 9, 8], BF16)
    opg1 = inp("opg1", [8, 1], F32)
    bbe1a = inp("bbe1a", [8, 1], F32)
    wg2t = inp("wg2t", [128, 9, 16], BF16)
    wbe2t = inp("wbe2t", [128, 9, 16], BF16)
    opg2 = inp("opg2", [16, 1], F32)
    bbe2a = inp("bbe2a", [16, 1], F32)
    wc1t = inp("wc1t", [8, 16], BF16)
    wc2t = inp("wc2t", [16, 1], BF16)
    b0 = inp("b0", [8, 1], F32)
    b1 = inp("b1", [16, 1], F32)
    b2 = inp("b2", [1, 1], F32)
    ones128f = inp("ones128f", [128, 1], F32)   # col of ones (reduce lhsT)
    onesbc = inp("onesbc", [16, 128], F32)      # ones block (reduce+broadcast)

    out_t = nc.declare_dram_parameter("out", [1, NPIX], F32, isOutput=True)

    with tile.TileContext(nc) as tc:
        with (
            tc.tile_pool(name="const", bufs=1) as cpool,
            tc.tile_pool(name="work", bufs=1) as wpool,
            tc.tile_pool(name="seg", bufs=3) as segpool,
            tc.tile_pool(name="dram", bufs=1, space="DRAM") as dpool,
        ):
            # ---------- load constants / inputs into SBUF ----------
            def load(pool, ap, dtype=None, name=None):
                t = pool.tile(list(ap.shape), dtype or ap.dtype, tag=name)
                nc.sync.dma_start(out=t[:], in_=ap[:])
                return t

            seg_sb = load(cpool, segbf, name="seg_sb")
            segval_sb = load(cpool, segval, name="segval_sb")
            rt_sb = load(cpool, rt, name="rt_sb")
            ct_sb = load(cpool, ct, name="ct_sb")
            ident_sb = load(cpool, ident, name="ident_sb")
            segsm_sb = load(cpool, segsm, name="segsm_sb")
            fsemT_sb = load(cpool, fsemT, name="fsemT_sb")
            x_sb = load(cpool, xq, name="x_sb")
            ones128f_sb = load(cpool, ones128f, name="ones128f_sb")
            onesbc_sb = load(cpool, onesbc, name="onesbc_sb")
            wsh3_sb = load(cpool, wsh3, name="wsh3_sb")
            bsh3_sb = load(cpool, bsh3, name="bsh3_sb")
            wg_sb = load(cpool, wg, name="wg_sb")
            wbe_sb = load(cpool, wbe, name="wbe_sb")
            opg0_sb = load(cpool, opg0, name="opg0_sb")
            bbe0a_sb = load(cpool, bbe0a, name="bbe0a_sb")
            wc0t_sb = load(cpool, wc0t, name="wc0t_sb")
            wg1t_sb = load(cpool, wg1t, name="wg1t_sb")
            wbe1t_sb = load(cpool, wbe1t, name="wbe1t_sb")
            opg1_sb = load(cpool, opg1, name="opg1_sb")
            bbe1a_sb = load(cpool, bbe1a, name="bbe1a_sb")
            wg2t_sb = load(cpool, wg2t, name="wg2t_sb")
            wbe2t_sb = load(cpool, wbe2t, name="wbe2t_sb")
            opg2_sb = load(cpool, opg2, name="opg2_sb")
            bbe2a_sb = load(cpool, bbe2a, name="bbe2a_sb")
            wc1t_sb = load(cpool, wc1t, name="wc1t_sb")
            wc2t_sb = load(cpool, wc2t, name="wc2t_sb")
            b0_sb = load(cpool, b0, name="b0_sb")
            b1_sb = load(cpool, b1, name="b1_sb")
            b2_sb = load(cpool, b2, name="b2_sb")

            # DRAM scratch
            crs_in = dpool.tile([SEGC + 1, PAYW], BF16)
            crs_out = dpool.tile([G, SEGC + 1, PAYW], BF16)
            cc0_in = dpool.tile([8, NPIX], F32)
            cc0_out = dpool.tile([8, NPIX], F32)

            # ---------- Phase A1: LayerNorm0 partial stats from x chunk ----------
            xsum = wpool.tile([128, 1], F32, tag="xsum")
            nc.vector.tensor_reduce(xsum[:], x_sb[:], mybir.AxisListType.XY, ALU.add)
            xsq_scratch = wpool.tile([128, 3, NPIX], BF16, tag="sq_scratch")
            xsumsq = wpool.tile([128, 1], F32, tag="xsumsq")
            nc.scalar.activation(xsq_scratch[:], x_sb[:], AOT.Square,
                                 accum_out=xsumsq[:])
            stats2 = wpool.tile([128, 2], F32, tag="stats2")
            nc.vector.tensor_copy(stats2[:, 0:1], xsum[:])
            nc.vector.tensor_copy(stats2[:, 1:2], xsumsq[:])

            psA_ctx = tc.tile_pool(name="psA", bufs=1, space="PSUM")
            psA = psA_ctx.__enter__()

            # cross-partition reduce of LN0 partial stats via ones-matmul
            stats1_ps = psA.tile([1, 2], F32, tag="stx")
            nc.tensor.matmul(stats1_ps[:], lhsT=ones128f_sb[:], rhs=stats2[:],
                             start=True, stop=True)
            stats1_sb = wpool.tile([1, 2], F32, tag="stats1_sb")
            nc.scalar.activation(stats1_sb[:], stats1_ps[:], AOT.Copy)
            # ride the AllGather payload as raw bits (bf16 view of f32)
            nc.sync.dma_start(out=crs_in[SEGC:SEGC + 1, 0:4],
                              in_=stats1_sb[:].bitcast(BF16))

            # ---------- Phase A2: segment averages for my 16 segments ----------
            ohsm = wpool.tile([128, SEGC, 8], BF16, tag="ohsm")
            for s in range(SEGC):
                nc.vector.tensor_scalar(ohsm[:, s, :], segsm_sb[:],
                                        segval_sb[:, s:s + 1], None, ALU.is_equal)
            sums_ps = psA.tile([SEGC, 385], F32, tag="sums")
            for c in range(8):
                nc.tensor.matmul(sums_ps[:], lhsT=ohsm[:, :, c],
                                 rhs=fsemT_sb[:, c, :],
                                 start=(c == 0), stop=(c == 7))
            sums_sb = wpool.tile([SEGC, 385], F32, tag="sums_sb")
            nc.scalar.activation(sums_sb[:], sums_ps[:], AOT.Copy)
            cnt_safe = wpool.tile([SEGC, 1], F32, tag="cnt_safe")
            nc.vector.tensor_scalar(cnt_safe[:], sums_sb[:, 384:385], 1.0, None,
                                    ALU.max)
            rec = wpool.tile([SEGC, 1], F32, tag="rec")
            nc.vector.reciprocal(rec[:], cnt_safe[:])
            mask = wpool.tile([SEGC, 1], F32, tag="mask")
            nc.vector.tensor_scalar(mask[:], sums_sb[:, 384:385], 0.5, None,
                                    ALU.is_gt)
            recm = wpool.tile([SEGC, 1], F32, tag="recm")
            nc.vector.tensor_mul(recm[:], rec[:], mask[:])
            avg_bf = wpool.tile([SEGC, CS], BF16, tag="avg_bf")
            nc.vector.tensor_scalar(avg_bf[:], sums_sb[:, 0:384], recm[:], None,
                                    ALU.mult)
            nc.sync.dma_start(out=crs_in[0:SEGC, 0:CS], in_=avg_bf[:])

            # ---------- Phase A3: M matrices for my 16 segments ----------
            mall_ps = psA.tile([32, SEGC * 32], F32, tag="mall")
            for s in range(SEGC):
                oh = segpool.tile([112, 4, 448], BF16, tag="oh")
                nc.vector.tensor_scalar(oh[:], seg_sb[:],
                                        segval_sb[0:112, s:s + 1], None,
                                        ALU.is_equal)
                a_ps = psA.tile([32, 448], F32, tag="aps", bufs=2)
                for c in range(4):
                    nc.tensor.matmul(a_ps[:], lhsT=rt_sb[:, c, :],
                                     rhs=oh[:, c, :],
                                     start=(c == 0), stop=(c == 3))
                a_sb = segpool.tile([32, 448], BF16, tag="asb")
                nc.scalar.activation(a_sb[:], a_ps[:], AOT.Copy)
                at_ps = psA.tile([112, 4, 32], BF16, tag="atps", bufs=2)
                for c in range(4):
                    nc.tensor.transpose(at_ps[:, c, :],
                                        a_sb[:, c * 112:(c + 1) * 112],
                                        ident_sb[:])
                at_sb = segpool.tile([112, 4, 32], BF16, tag="atsb")
                nc.scalar.activation(at_sb[:], at_ps[:], AOT.Copy)
                for c in range(4):
                    nc.tensor.matmul(mall_ps[:, s * 32:(s + 1) * 32],
                                     lhsT=at_sb[:, c, :], rhs=ct_sb[:, c, :],
                                     start=(c == 0), stop=(c == 3))

            # M [32(p), 16(s), 32(q)] -> payload rows [16(s), p*32+q]
            mall_bf = wpool.tile([32, SEGC, 32], BF16, tag="mall_bf")
            nc.vector.tensor_copy(
                mall_bf[:], mall_ps[:].rearrange("p (s q) -> p s q", s=SEGC))
            nc.sync.dma_start(
                out=crs_in[0:SEGC, CS:PAYW].rearrange(
                    "s (p q) -> p s q", p=32),
                in_=mall_bf[:])

            psA_ctx.__exit__(None, None, None)
            psB_ctx = tc.tile_pool(name="psB", bufs=1, space="PSUM")
            psB = psB_ctx.__enter__()

            # ---------- C1: ONE small AllGather of (avg | M | stats) ----------
            nc.gpsimd.collective_compute(
                "AllGather", ALU.bypass,
                replica_groups=[[0, 1, 2, 3], [4, 5, 6, 7]],
                ins=[crs_in[:]], outs=[crs_out[:]],
            )

            # ---------- Phase B0: unpack gather + LN0 scalars ----------
            avg_all = wpool.tile([NSEG, CS], BF16, tag="avg_all")
            m_all = wpool.tile([NSEG, NPIX], BF16, tag="m_all")
            for g in range(G):
                nc.sync.dma_start(out=avg_all[g * SEGC:(g + 1) * SEGC, :],
                                  in_=crs_out[g, 0:SEGC, 0:CS])
                nc.sync.dma_start(out=m_all[g * SEGC:(g + 1) * SEGC, :],
                                  in_=crs_out[g, 0:SEGC, CS:PAYW])
            stats4 = wpool.tile([G, 4], BF16, tag="stats4")
            nc.sync.dma_start(out=stats4[:], in_=crs_out[:, SEGC, 0:4])

            # reduce the 4 partial stats AND broadcast to 128 partitions in
            # one ones-matmul
            st0_ps = psB.tile([128, 2], F32, tag="acc", bufs=2)
            nc.tensor.matmul(st0_ps[:], lhsT=onesbc_sb[0:G, :],
                             rhs=stats4[:].bitcast(F32), start=True, stop=True)
            st0_bc = wpool.tile([128, 2], F32, tag="st0_bc")
            nc.scalar.activation(st0_bc[:], st0_ps[:], AOT.Copy)

            def ln_from_bc(st_bc, n_elems, nparts, tag):
                """st_bc [nparts,2]=(sum,sumsq) replicated -> mu,istd."""
                ms = wpool.tile([nparts, 2], F32, tag=tag + "_ms")
                nc.vector.tensor_scalar(ms[:], st_bc[:], 1.0 / n_elems, None,
                                        ALU.mult)
                musq = wpool.tile([nparts, 1], F32, tag=tag + "_musq")
                nc.vector.tensor_mul(musq[:], ms[:, 0:1], ms[:, 0:1])
                var = wpool.tile([nparts, 1], F32, tag=tag + "_var")
                nc.vector.tensor_sub(var[:], ms[:, 1:2], musq[:])
                vare = wpool.tile([nparts, 1], F32, tag=tag + "_vare")
                nc.vector.tensor_scalar(vare[:], var[:], EPS, None, ALU.add)
                lnv = wpool.tile([nparts, 1], F32, tag=tag + "_lnv")
                nc.scalar.activation(lnv[:], vare[:], AOT.Ln)
                istd = wpool.tile([nparts, 1], F32, tag=tag + "_istd")
                nc.scalar.activation(istd[:], lnv[:], AOT.Exp, scale=-0.5)
                return ms[:, 0:1], istd

            mu0_bc, istd0_bc = ln_from_bc(st0_bc, LN0_N, 128, "ln0")

            # ---------- Phase B1: full sem_rs, padded, in SBUF (bf16) ----------
            semrs_pad = []
            for k in range(3):
                sp = wpool.tile([128, 34, 34], BF16, tag=f"semrs_pad{k}")
                nc.vector.memset(sp[:], 0.0)
                for h in range(2):
                    ps = psB.tile([128, 512], F32, tag="big0", bufs=4)
                    nc.tensor.matmul(ps[:],
                                     lhsT=avg_all[:, k * 128:(k + 1) * 128],
                                     rhs=m_all[:, h * 512:(h + 1) * 512],
                                     start=True, stop=True)
                    nc.scalar.activation(
                        sp[:, 1 + h * 16:17 + h * 16, 1:33],
                        ps[:].rearrange("c (r w) -> c r w", r=16), AOT.Copy)
                semrs_pad.append(sp)

            # ---------- shared 3x3 convs (replicated; L=0 now, 1/2 later) ----
            hsh_pad = [wpool.tile([128, 34, 34], BF16, tag=f"hsh_pad{L}",
                                  name=f"hsh_pad{L}")
                       for L in range(3)]

            def build_hsh(L):
                hp = hsh_pad[L]
                nc.vector.memset(hp[:], 0.0)
                for h in range(2):
                    ps = psB.tile([128, 512], F32, tag="big0", bufs=4)
                    first = True
                    for cic in range(3):
                        for t in range(9):
                            dy, dx = t // 3, t % 3
                            nc.tensor.matmul(
                                ps[:].rearrange("c (r w) -> c r w", r=16),
                                lhsT=wsh3_sb[:, L, t, cic, :],
                                rhs=_conv_windows(semrs_pad[cic][:], 16, 32,
                                                  dy, dx, row0=h * 16),
                                start=first, stop=(cic == 2 and t == 8))
                            first = False
                    nc.scalar.activation(
                        hp[:, 1 + h * 16:17 + h * 16, 1:33],
                        ps[:].rearrange("c (r w) -> c r w", r=16), AOT.Relu,
                        bias=bsh3_sb[:, L:L + 1])

            build_hsh(0)

            # ---------- Phase B5/B6: layer-0 gamma/beta convs + SPADE + c0 ----
            c0_ps = [psB.tile([8, 512], F32, tag="acc", bufs=2,
                              name=f"c0_ps{h}") for h in range(2)]
            for m in range(3):
                xn_m = wpool.tile([128, NPIX], BF16, tag="xn", bufs=2)
                nc.vector.tensor_scalar(xn_m[:], x_sb[:, m, :], mu0_bc[:],
                                        istd0_bc[:], ALU.subtract, ALU.mult)
                for h in range(2):
                    gb_ps = {}
                    for name, w_sb in (("g", wg_sb), ("be", wbe_sb)):
                        ps = psB.tile([128, 512], F32,
                                      tag=("big0" if name == "g" else "big1"),
                                      bufs=(4 if name == "g" else 2))
                        for t in range(9):
                            dy, dx = t // 3, t % 3
                            nc.tensor.matmul(
                                ps[:].rearrange("c (r w) -> c r w", r=16),
                                lhsT=w_sb[:, t, m * 128:(m + 1) * 128],
                                rhs=_conv_windows(hsh_pad[0][:], 16, 32, dy, dx,
                                                  row0=h * 16),
                                start=(t == 0), stop=(t == 8))
                        gb_ps[name] = ps
                    # spade: out = xn*(1+gamma+bg) + (beta+bbe)
                    hs = slice(h * 512, (h + 1) * 512)
                    u = wpool.tile([128, 512], BF16, tag="spade_u", bufs=2)
                    nc.vector.scalar_tensor_tensor(u[:], gb_ps["g"][:],
                                                   opg0_sb[:, m:m + 1],
                                                   xn_m[:, hs],
                                                   ALU.add, ALU.mult)
                    sp0 = wpool.tile([128, 512], BF16, tag="spade_o", bufs=2)
                    nc.vector.scalar_tensor_tensor(sp0[:], gb_ps["be"][:],
                                                   bbe0a_sb[:, m:m + 1], u[:],
                                                   ALU.add, ALU.add)
                    nc.tensor.matmul(c0_ps[h][:], lhsT=wc0t_sb[:, m, :],
                                     rhs=sp0[:], start=(m == 0), stop=(m == 2))

            c0p_sb = wpool.tile([8, NPIX], F32, tag="f32buf")
            for h in range(2):
                nc.scalar.activation(c0p_sb[:, h * 512:(h + 1) * 512],
                                     c0_ps[h][:], AOT.Copy)
            nc.sync.dma_start(out=cc0_in[:], in_=c0p_sb[:])
            nc.gpsimd.collective_compute(
                "AllReduce", ALU.add,
                replica_groups=[[0, 1, 2, 3], [4, 5, 6, 7]],
                ins=[cc0_in[:]], outs=[cc0_out[:]],
            )

            # emitted after the collective trigger: overlaps C2 on the PE
            build_hsh(1)
            build_hsh(2)

            # ---------- Phase B7: h1 + LN1 ----------
            def softplus_to(dst, src_ap, bias_sb, nparts, tag):
                """dst = ln(1 + exp(src + bias)); inputs here are small, so
                exp cannot overflow and both ACTs share one table set."""
                e = wpool.tile([nparts, NPIX], F32, tag=tag + "_e")
                nc.scalar.activation(e[:], src_ap, AOT.Exp, bias=bias_sb[:])
                nc.scalar.activation(dst[:], e[:], AOT.Ln, bias=1.0)

            c0_sb = wpool.tile([8, NPIX], F32, tag="f32buf2")
            nc.sync.dma_start(out=c0_sb[:], in_=cc0_out[:])
            h1_f32 = wpool.tile([8, NPIX], F32, tag="hbuf")
            softplus_to(h1_f32, c0_sb[:], b0_sb, 8, "sp1")

            def ln_small(h_f32, nparts, n_elems, tag):
                hsum = wpool.tile([nparts, 1], F32, tag=tag + "_hsum")
                nc.vector.tensor_reduce(hsum[:], h_f32[:], mybir.AxisListType.X,
                                        ALU.add)
                hsq = wpool.tile([nparts, NPIX], BF16, tag="sq_scratch2")
                hsumsq = wpool.tile([nparts, 1], F32, tag=tag + "_hsumsq")
                nc.scalar.activation(hsq[:], h_f32[:], AOT.Square,
                                     accum_out=hsumsq[:])
                st2 = wpool.tile([nparts, 2], F32, tag=tag + "_st2")
                nc.vector.tensor_copy(st2[:, 0:1], hsum[:])
                nc.vector.tensor_copy(st2[:, 1:2], hsumsq[:])
                st_ps = psB.tile([128, 2], F32, tag="acc", bufs=2)
                nc.tensor.matmul(st_ps[:], lhsT=onesbc_sb[0:nparts, :],
                                 rhs=st2[:], start=True, stop=True)
                st_bc = wpool.tile([nparts, 2], F32, tag=tag + "_stbc")
                nc.scalar.activation(st_bc[:], st_ps[0:nparts, :], AOT.Copy)
                return ln_from_bc(st_bc, n_elems, nparts, tag)

            mu1_bc, istd1_bc = ln_small(h1_f32, 8, LN1_N, "ln1")
            xn1 = wpool.tile([8, NPIX], BF16, tag="xn1")
            nc.vector.tensor_scalar(xn1[:], h1_f32[:], mu1_bc[:], istd1_bc[:],
                                    ALU.subtract, ALU.mult)

            # ---------- Phase B8: layers 1 and 2 (replicated) ----------
            def spade_small(xn_bf, nco, wgt_sb, opg_sb, wbet_sb, bbea_sb, tag):
                sp_ = wpool.tile([nco, NPIX], BF16, tag=f"{tag}_sp")
                pad_img = hsh_pad[1 if tag == "l1" else 2]
                for h in range(2):
                    ps = {}
                    for name, w_sb in (("g", wgt_sb), ("be", wbet_sb)):
                        p = psB.tile([nco, 512], F32,
                                     tag=("big0" if name == "g" else "big1"),
                                     bufs=(4 if name == "g" else 2))
                        for t in range(9):
                            dy, dx = t // 3, t % 3
                            nc.tensor.matmul(
                                p[:].rearrange("c (r w) -> c r w", r=16),
                                lhsT=w_sb[:, t, :],
                                rhs=_conv_windows(pad_img[:], 16, 32, dy, dx,
                                                  row0=h * 16),
                                start=(t == 0), stop=(t == 8))
                        ps[name] = p
                    hs = slice(h * 512, (h + 1) * 512)
                    u_ = wpool.tile([nco, 512], BF16, tag=f"{tag}_u")
                    nc.vector.scalar_tensor_tensor(u_[:], ps["g"][:], opg_sb[:],
                                                   xn_bf[:, hs],
                                                   ALU.add, ALU.mult)
                    nc.vector.scalar_tensor_tensor(sp_[:, hs], ps["be"][:],
                                                   bbea_sb[:], u_[:],
                                                   ALU.add, ALU.add)
                return sp_

            sp1 = spade_small(xn1, 8, wg1t_sb, opg1_sb, wbe1t_sb, bbe1a_sb, "l1")
            c1_sb = wpool.tile([16, NPIX], F32, tag="f32buf")
            for h in range(2):
                c1_ps = psB.tile([16, 512], F32, tag="acc", bufs=2)
                nc.tensor.matmul(c1_ps[:], lhsT=wc1t_sb[:],
                                 rhs=sp1[:, h * 512:(h + 1) * 512],
                                 start=True, stop=True)
                nc.scalar.activation(c1_sb[:, h * 512:(h + 1) * 512], c1_ps[:],
                                     AOT.Copy)
            h2_f32 = wpool.tile([16, NPIX], F32, tag="hbuf2")
            softplus_to(h2_f32, c1_sb[:], b1_sb, 16, "sp2")

            mu2_bc, istd2_bc = ln_small(h2_f32, 16, LN2_N, "ln2")
            xn2 = wpool.tile([16, NPIX], BF16, tag="xn2")
            nc.vector.tensor_scalar(xn2[:], h2_f32[:], mu2_bc[:], istd2_bc[:],
                                    ALU.subtract, ALU.mult)

            sp2 = spade_small(xn2, 16, wg2t_sb, opg2_sb, wbe2t_sb, bbe2a_sb, "l2")
            c2_sb = wpool.tile([1, NPIX], F32, tag="f32buf3")
            for h in range(2):
                c2_ps = psB.tile([1, 512], F32, tag="acc", bufs=2)
                nc.tensor.matmul(c2_ps[:], lhsT=wc2t_sb[:],
                                 rhs=sp2[:, h * 512:(h + 1) * 512],
                                 start=True, stop=True)
                nc.scalar.activation(c2_sb[:, h * 512:(h + 1) * 512], c2_ps[:],
                                     AOT.Copy)
            out_f32 = wpool.tile([1, NPIX], F32, tag="hbuf3")
            softplus_to(out_f32, c2_sb[:], b2_sb, 1, "sp3")
            nc.sync.dma_start(out=out_t[:], in_=out_f32[:])
            psB_ctx.__exit__(None, None, None)

    nc.finalize()
    _split_sync_waits(nc)
    return nc


def _pack_inputs(inputs):
    f32 = np.float32
    R = _resize_matrix(HI, HP)      # [32, 448]
    C = _resize_matrix(WI, WP)      # [32, 448]
    rt = np.zeros((112, 4, 32), f32)
    ctm = np.zeros((112, 4, 32), f32)
    for c in range(4):
        rt[:, c, :] = R[:, c * 112:(c + 1) * 112].T
        ctm[:, c, :] = C[:, c * 112:(c + 1) * 112].T
    ident = np.eye(32, dtype=f32)

    segmap = inputs["segmap"]            # [2, 448, 448] int32
    f_sem = inputs["f_semantic"]         # [2, 384, 32, 32]
    x_main = inputs["x_main"]            # [2, 1536, 32, 32]
    rows = (np.arange(HP) * HI) // HP

    def tap_t(w):  # [co, ci, 3, 3] -> [ci, 9, co]
        return np.ascontiguousarray(w.transpose(1, 2, 3, 0).reshape(
            w.shape[1], 9, w.shape[0]))

    # all three shared conv layers: [ci_p, L, tap, cic, co]
    wsh3 = np.stack([
        tap_t(inputs[f"w_sh{L}"]).reshape(3, 128, 9, HM).transpose(1, 2, 0, 3)
        for L in range(3)], axis=1).astype(BF16_NP)
    bsh3 = np.stack([inputs[f"b_sh{L}"] for L in range(3)], axis=1).astype(f32)

    maps = []
    for cid in range(8):
        b, g = cid // G, cid % G
        d = {}
        seg = segmap[b].astype(f32)
        d["segbf"] = seg.reshape(4, 112, WI).transpose(1, 0, 2).astype(BF16_NP)
        d["segval"] = np.broadcast_to(
            (np.arange(SEGC, dtype=f32) + SEGC * g)[None, :], (128, SEGC)
        ).astype(f32).copy()
        d["rt"] = rt.astype(BF16_NP)
        d["ct"] = ctm.astype(BF16_NP)
        d["ident"] = ident.astype(BF16_NP)
        seg_small = seg[rows[:, None], rows[None, :]].reshape(-1)   # [1024]
        d["segsm"] = seg_small.reshape(8, 128).T.astype(BF16_NP).copy()
        fT = f_sem[b].reshape(CS, NPIX).T                           # [1024, 384]
        fTe = np.concatenate([fT, np.ones((NPIX, 1), f32)], 1)      # [1024, 385]
        d["fsemT"] = fTe.reshape(8, 128, 385).transpose(1, 0, 2).astype(
            BF16_NP).copy()
        xc = x_main[b, g * COC:(g + 1) * COC].reshape(COC, NPIX)
        d["xq"] = np.ascontiguousarray(
            xc.reshape(3, 128, NPIX).transpose(1, 0, 2))
        d["wsh3"] = wsh3
        d["bsh3"] = bsh3
        cosl = slice(g * COC, (g + 1) * COC)
        d["wg"] = tap_t(inputs["w_g0"][cosl]).astype(BF16_NP)      # [128,9,384]
        d["wbe"] = tap_t(inputs["w_be0"][cosl]).astype(BF16_NP)
        d["opg0"] = np.ascontiguousarray(
            (1.0 + inputs["b_g0"][cosl]).reshape(3, 128).T).astype(f32)
        d["bbe0a"] = np.ascontiguousarray(
            inputs["b_be0"][cosl].reshape(3, 128).T).astype(f32)
        wc0 = inputs["w_c0"][:, :, 0, 0]                           # [8, 1536]
        d["wc0t"] = np.ascontiguousarray(
            wc0[:, cosl].T.reshape(3, 128, 8).transpose(1, 0, 2)).astype(BF16_NP)
        d["wg1t"] = tap_t(inputs["w_g1"]).astype(BF16_NP)          # [128,9,8]
        d["wbe1t"] = tap_t(inputs["w_be1"]).astype(BF16_NP)
        d["opg1"] = (1.0 + inputs["b_g1"]).reshape(8, 1).astype(f32)
        d["bbe1a"] = inputs["b_be1"].reshape(8, 1).astype(f32)
        d["wg2t"] = tap_t(inputs["w_g2"]).astype(BF16_NP)          # [128,9,16]
        d["wbe2t"] = tap_t(inputs["w_be2"]).astype(BF16_NP)
        d["opg2"] = (1.0 + inputs["b_g2"]).reshape(16, 1).astype(f32)
        d["bbe2a"] = inputs["b_be2"].reshape(16, 1).astype(f32)
        d["wc1t"] = inputs["w_c1"][:, :, 0, 0].T.astype(BF16_NP).copy()  # [8,16]
        d["wc2t"] = inputs["w_c2"][:, :, 0, 0].T.astype(BF16_NP).copy()  # [16,1]
        d["b0"] = inputs["bias0"].reshape(8, 1).astype(f32)
        d["b1"] = inputs["bias1"].reshape(16, 1).astype(f32)
        d["b2"] = inputs["bias2"].reshape(1, 1).astype(f32)
        d["ones128f"] = np.ones((128, 1), f32)
        d["onesbc"] = np.ones((16, 128), f32)
        maps.append(d)
    return maps


def kernel(**inputs):
    if "nc" not in _NC_CACHE:
        _NC_CACHE["nc"] = _build_nc()
    nc = _NC_CACHE["nc"]
    in_maps = _pack_inputs(inputs)
    res = run_bass_kernel_spmd(nc, in_maps, list(range(8)))
    out = np.zeros((B, 1, HP, WP), np.float32)
    out[0, 0] = res.results[0]["out"].reshape(HP, WP)
    out[1, 0] = res.results[4]["out"].reshape(HP, WP)
    return out


if __name__ == "__main__":
    nc = _build_nc()
    print("built OK; instructions:",
          sum(len(b.instructions) for f in nc.m.functions for b in f.blocks))


# revision 19
# speedup vs baseline: 1.3363x; 1.0314x over previous
"""Trainium2 Bass kernel for nn_DinoGazeSpade (segment_reduce + SPADE stack).

Strategy (8 NeuronCores, SPMD single program):
  - Two groups of 4 cores; group = batch index b (0..1), rank g = core % 4.
  - Painted-map + bilinear resize is reformulated as segment matrices:
        sem_rs[c,p,q] = sum_s avg[s,c] * M[s,p,q],
        M[s] = R @ onehot_s @ C^T   (R, C: 32x448 separable resize matrices)
    Each core builds avg + M for its 16 segments only, then ONE small
    in-group AllGather ships (avg ‖ M ‖ LN0 stats) [17,1408] bf16 (~48KB);
    every core then computes the FULL sem_rs with a cheap k=64 matmul set.
  - The three shared 3x3 convs (w_sh*) are fully replicated (no AllGather):
    sh0 right after sem_rs; sh1/sh2 are emitted after the c0 AllReduce
    trigger so their matmuls overlap the collective.
  - gamma/beta convs of layer 0 are split by output channel (384 per core);
    the pointwise c0 conv partials are AllReduced (C2).  Layers 1-2 are tiny
    and replicated.
  - conv3x3 = 9 shifted matmuls over a zero-padded [C,34,34] SBUF image.
  - LayerNorm scalars use a ones-matmul reduce+broadcast (no DRAM roundtrip)
    and Rsqrt; softplus is the native ACT Softplus.

The host side packs per-core shards / weight transposes (layout only) and
reassembles the [2,1,32,32] output from cores 0 and 4.
"""

import numpy as np

from concourse import bass, tile, mybir
from concourse.bass_utils import run_bass_kernel_spmd

F32 = mybir.dt.float32
BF16 = mybir.dt.bfloat16
BF16_NP = mybir.dt.np(BF16)
AOT = mybir.ActivationFunctionType
ALU = mybir.AluOpType

# Problem dims
B, CM, CS, HP, WP, HI, WI, HM, NSEG = 2, 1536, 384, 32, 32, 448, 448, 128, 64
G = 4              # cores per batch group
SEGC = NSEG // G   # segments per core = 16
COC = CM // G      # gamma/beta out-channel chunk per core = 384
NPIX = HP * WP     # 1024
EPS = 1e-12
LN0_N = float(CM * NPIX)
LN1_N = float(8 * NPIX)
LN2_N = float(16 * NPIX)
PAYW = CS + NPIX   # 1408 payload cols: avg | M

_NC_CACHE = {}


def _resize_matrix(n_in, n_out):
    """Row matrix of jax.image.resize(..., 'bilinear') for downsampling
    (antialiased triangle kernel, normalized rows). Verified vs jax."""
    scale = n_out / n_in
    p = np.arange(n_out, dtype=np.float64)[:, None]
    i = np.arange(n_in, dtype=np.float64)[None, :]
    center = (p + 0.5) / scale - 0.5
    w = np.maximum(0.0, 1.0 - np.abs(i - center) * scale)
    w = w / w.sum(axis=1, keepdims=True)
    return w.astype(np.float32)


def _split_sync_waits(nc, max_waits=1):
    """walrus in this container encodes at most one sync-wait per
    instruction; hoist extras onto preceding same-engine NoOps."""
    n = 0
    for fn in nc.m.functions:
        for blk in fn.blocks:
            new_insts = []
            for inst in blk.instructions:
                si = getattr(inst, "sync_info", None)
                if si is not None and si.on_wait and len(si.on_wait) > max_waits:
                    waits = list(si.on_wait)
                    head, rest = waits[:-max_waits], waits[-max_waits:]
                    for i in range(0, len(head), max_waits):
                        new_insts.append(mybir.InstNoOp(
                            name=f"I-ws-{nc.next_id()}", engine=inst.engine,
                            ins=[], outs=[],
                            sync_info=mybir.SyncInfo(
                                on_wait=list(head[i:i + max_waits]), on_update=[]),
                        ))
                    si.on_wait = rest
                    n += 1
                new_insts.append(inst)
            blk.instructions = new_insts
    return n


def _conv_windows(pad_ap, rows, cols, dy, dx, row0=0):
    """AP view [P, rows, cols] of a padded [P, 34, 34] image at tap (dy,dx)."""
    return pad_ap[:, row0 + dy:row0 + dy + rows, dx:dx + cols]


def _build_nc():
    nc = bass.Bass()

    def inp(name, shape, dtype):
        return nc.declare_dram_parameter(name, list(shape), dtype, isOutput=False)

    # --- inputs (per-core packed shards; see _pack_inputs) ---
    segbf = inp("segbf", [112, 4, 448], BF16)
    segval = inp("segval", [128, SEGC], F32)
    rt = inp("rt", [112, 4, 32], BF16)
    ct = inp("ct", [112, 4, 32], BF16)
    ident = inp("ident", [32, 32], BF16)
    segsm = inp("segsm", [128, 8], BF16)
    fsemT = inp("fsemT", [128, 8, 385], BF16)
    xq = inp("xq", [128, 3, NPIX], F32)
    wsh3 = inp("wsh3", [128, 3, 9, 3, 128], BF16)  # all sh: [ci_p, L, tap, cic, co]
    bsh3 = inp("bsh3", [128, 3], F32)              # all b_sh: [co, L]
    wg = inp("wg", [128, 9, COC], BF16)            # w_g0 chunk:  [ci, tap, co_local]
    wbe = inp("wbe", [128, 9, COC], BF16)
    opg0 = inp("opg0", [128, 3], F32)      # 1 + b_g0 chunk, [ci_p, m]
    bbe0a = inp("bbe0a", [128, 3], F32)    # b_be0 chunk, [ci_p, m]
    wc0t = inp("wc0t", [128, 3, 8], BF16)
    wgbe1 = inp("wgbe1", [128, 9, 40], BF16)   # gamma @0:8, beta @32:40
    opg1 = inp("opg1", [8, 1], F32)
    bbe1a = inp("bbe1a", [8, 1], F32)
    wgbe2 = inp("wgbe2", [128, 9, 48], BF16)   # gamma @0:16, beta @32:48
    opg2 = inp("opg2", [16, 1], F32)
    bbe2a = inp("bbe2a", [16, 1], F32)
    wc1t = inp("wc1t", [8, 16], BF16)
    wc2t = inp("wc2t", [16, 1], BF16)
    b0 = inp("b0", [8, 1], F32)
    b1 = inp("b1", [16, 1], F32)
    b2 = inp("b2", [1, 1], F32)
    ones128f = inp("ones128f", [128, 1], F32)   # col of ones (reduce lhsT)
    onesbc = inp("onesbc", [16, 128], F32)      # ones block (reduce+broadcast)

    out_t = nc.declare_dram_parameter("out", [1, NPIX], F32, isOutput=True)

    with tile.TileContext(nc) as tc:
        with (
            tc.tile_pool(name="const", bufs=1) as cpool,
            tc.tile_pool(name="work", bufs=1) as wpool,
            tc.tile_pool(name="seg", bufs=3) as segpool,
            tc.tile_pool(name="dram", bufs=1, space="DRAM") as dpool,
        ):
            # ---------- load constants / inputs into SBUF ----------
            def load(pool, ap, dtype=None, name=None):
                t = pool.tile(list(ap.shape), dtype or ap.dtype, tag=name)
                nc.sync.dma_start(out=t[:], in_=ap[:])
                return t

            seg_sb = load(cpool, segbf, name="seg_sb")
            segval_sb = load(cpool, segval, name="segval_sb")
            rt_sb = load(cpool, rt, name="rt_sb")
            ct_sb = load(cpool, ct, name="ct_sb")
            ident_sb = load(cpool, ident, name="ident_sb")
            segsm_sb = load(cpool, segsm, name="segsm_sb")
            fsemT_sb = load(cpool, fsemT, name="fsemT_sb")
            x_sb = load(cpool, xq, name="x_sb")
            ones128f_sb = load(cpool, ones128f, name="ones128f_sb")
            onesbc_sb = load(cpool, onesbc, name="onesbc_sb")
            wsh3_sb = load(cpool, wsh3, name="wsh3_sb")
            bsh3_sb = load(cpool, bsh3, name="bsh3_sb")
            wg_sb = load(cpool, wg, name="wg_sb")
            wbe_sb = load(cpool, wbe, name="wbe_sb")
            opg0_sb = load(cpool, opg0, name="opg0_sb")
            bbe0a_sb = load(cpool, bbe0a, name="bbe0a_sb")
            wc0t_sb = load(cpool, wc0t, name="wc0t_sb")
            wgbe1_sb = load(cpool, wgbe1, name="wgbe1_sb")
            opg1_sb = load(cpool, opg1, name="opg1_sb")
            bbe1a_sb = load(cpool, bbe1a, name="bbe1a_sb")
            wgbe2_sb = load(cpool, wgbe2, name="wgbe2_sb")
            opg2_sb = load(cpool, opg2, name="opg2_sb")
            bbe2a_sb = load(cpool, bbe2a, name="bbe2a_sb")
            wc1t_sb = load(cpool, wc1t, name="wc1t_sb")
            wc2t_sb = load(cpool, wc2t, name="wc2t_sb")
            b0_sb = load(cpool, b0, name="b0_sb")
            b1_sb = load(cpool, b1, name="b1_sb")
            b2_sb = load(cpool, b2, name="b2_sb")

            # DRAM scratch
            crs_in = dpool.tile([SEGC + 1, PAYW], BF16)
            crs_out = dpool.tile([G, SEGC + 1, PAYW], BF16)
            cc0_in = dpool.tile([8, NPIX], F32)
            cc0_out = dpool.tile([8, NPIX], F32)

            # ---------- Phase A1: LayerNorm0 partial stats from x chunk ----------
            xsum = wpool.tile([128, 1], F32, tag="xsum")
            nc.vector.tensor_reduce(xsum[:], x_sb[:], mybir.AxisListType.XY, ALU.add)
            xsq_scratch = wpool.tile([128, 3, NPIX], BF16, tag="sq_scratch")
            xsumsq = wpool.tile([128, 1], F32, tag="xsumsq")
            nc.scalar.activation(xsq_scratch[:], x_sb[:], AOT.Square,
                                 accum_out=xsumsq[:])
            stats2 = wpool.tile([128, 2], F32, tag="stats2")
            nc.vector.tensor_copy(stats2[:, 0:1], xsum[:])
            nc.vector.tensor_copy(stats2[:, 1:2], xsumsq[:])

            psA_ctx = tc.tile_pool(name="psA", bufs=1, space="PSUM")
            psA = psA_ctx.__enter__()

            # cross-partition reduce of LN0 partial stats via ones-matmul
            stats1_ps = psA.tile([1, 2], F32, tag="stx")
            nc.tensor.matmul(stats1_ps[:], lhsT=ones128f_sb[:], rhs=stats2[:],
                             start=True, stop=True)
            stats1_sb = wpool.tile([1, 2], F32, tag="stats1_sb")
            nc.scalar.activation(stats1_sb[:], stats1_ps[:], AOT.Copy)
            # ride the AllGather payload as raw bits (bf16 view of f32)
            nc.sync.dma_start(out=crs_in[SEGC:SEGC + 1, 0:4],
                              in_=stats1_sb[:].bitcast(BF16))

            # ---------- Phase A2: segment averages for my 16 segments ----------
            ohsm = wpool.tile([128, SEGC, 8], BF16, tag="ohsm")
            for s in range(SEGC):
                nc.vector.tensor_scalar(ohsm[:, s, :], segsm_sb[:],
                                        segval_sb[:, s:s + 1], None, ALU.is_equal)
            sums_ps = psA.tile([SEGC, 385], F32, tag="sums")
            for c in range(8):
                nc.tensor.matmul(sums_ps[:], lhsT=ohsm[:, :, c],
                                 rhs=fsemT_sb[:, c, :],
                                 start=(c == 0), stop=(c == 7))
            sums_sb = wpool.tile([SEGC, 385], F32, tag="sums_sb")
            nc.scalar.activation(sums_sb[:], sums_ps[:], AOT.Copy)
            cnt_safe = wpool.tile([SEGC, 1], F32, tag="cnt_safe")
            nc.vector.tensor_scalar(cnt_safe[:], sums_sb[:, 384:385], 1.0, None,
                                    ALU.max)
            rec = wpool.tile([SEGC, 1], F32, tag="rec")
            nc.vector.reciprocal(rec[:], cnt_safe[:])
            mask = wpool.tile([SEGC, 1], F32, tag="mask")
            nc.vector.tensor_scalar(mask[:], sums_sb[:, 384:385], 0.5, None,
                                    ALU.is_gt)
            recm = wpool.tile([SEGC, 1], F32, tag="recm")
            nc.vector.tensor_mul(recm[:], rec[:], mask[:])
            avg_bf = wpool.tile([SEGC, CS], BF16, tag="avg_bf")
            nc.vector.tensor_scalar(avg_bf[:], sums_sb[:, 0:384], recm[:], None,
                                    ALU.mult)
            nc.sync.dma_start(out=crs_in[0:SEGC, 0:CS], in_=avg_bf[:])

            # ---------- Phase A3: M matrices for my 16 segments ----------
            mall_ps = psA.tile([32, SEGC * 32], F32, tag="mall")
            for s in range(SEGC):
                oh = segpool.tile([112, 4, 448], BF16, tag="oh")
                nc.vector.tensor_scalar(oh[:], seg_sb[:],
                                        segval_sb[0:112, s:s + 1], None,
                                        ALU.is_equal)
                a_ps = psA.tile([32, 448], F32, tag="aps", bufs=2)
                for c in range(4):
                    nc.tensor.matmul(a_ps[:], lhsT=rt_sb[:, c, :],
                                     rhs=oh[:, c, :],
                                     start=(c == 0), stop=(c == 3))
                a_sb = segpool.tile([32, 448], BF16, tag="asb")
                nc.scalar.activation(a_sb[:], a_ps[:], AOT.Copy)
                at_ps = psA.tile([112, 4, 32], BF16, tag="atps", bufs=2)
                for c in range(4):
                    nc.tensor.transpose(at_ps[:, c, :],
                                        a_sb[:, c * 112:(c + 1) * 112],
                                        ident_sb[:])
                at_sb = segpool.tile([112, 4, 32], BF16, tag="atsb")
                nc.scalar.activation(at_sb[:], at_ps[:], AOT.Copy)
                for c in range(4):
                    nc.tensor.matmul(mall_ps[:, s * 32:(s + 1) * 32],
                                     lhsT=at_sb[:, c, :], rhs=ct_sb[:, c, :],
                                     start=(c == 0), stop=(c == 3))

            # M [32(p), 16(s), 32(q)] -> payload rows [16(s), p*32+q]
            mall_bf = wpool.tile([32, SEGC, 32], BF16, tag="mall_bf")
            nc.vector.tensor_copy(
                mall_bf[:], mall_ps[:].rearrange("p (s q) -> p s q", s=SEGC))
            nc.sync.dma_start(
                out=crs_in[0:SEGC, CS:PAYW].rearrange(
                    "s (p q) -> p s q", p=32),
                in_=mall_bf[:])

            psA_ctx.__exit__(None, None, None)
            psB_ctx = tc.tile_pool(name="psB", bufs=1, space="PSUM")
            psB = psB_ctx.__enter__()

            # ---------- C1: ONE small AllGather of (avg | M | stats) ----------
            nc.gpsimd.collective_compute(
                "AllGather", ALU.bypass,
                replica_groups=[[0, 1, 2, 3], [4, 5, 6, 7]],
                ins=[crs_in[:]], outs=[crs_out[:]],
            )

            # ---------- Phase B0: unpack gather + LN0 scalars ----------
            avg_all = wpool.tile([NSEG, CS], BF16, tag="avg_all")
            m_all = wpool.tile([NSEG, NPIX], BF16, tag="m_all")
            for g in range(G):
                nc.sync.dma_start(out=avg_all[g * SEGC:(g + 1) * SEGC, :],
                                  in_=crs_out[g, 0:SEGC, 0:CS])
                nc.sync.dma_start(out=m_all[g * SEGC:(g + 1) * SEGC, :],
                                  in_=crs_out[g, 0:SEGC, CS:PAYW])
            stats4 = wpool.tile([G, 4], BF16, tag="stats4")
            nc.sync.dma_start(out=stats4[:], in_=crs_out[:, SEGC, 0:4])

            # reduce the 4 partial stats AND broadcast to 128 partitions in
            # one ones-matmul
            st0_ps = psB.tile([128, 2], F32, tag="acc", bufs=2)
            nc.tensor.matmul(st0_ps[:], lhsT=onesbc_sb[0:G, :],
                             rhs=stats4[:].bitcast(F32), start=True, stop=True)
            st0_bc = wpool.tile([128, 2], F32, tag="st0_bc")
            nc.scalar.activation(st0_bc[:], st0_ps[:], AOT.Copy)

            def ln_from_bc(st_bc, n_elems, nparts, tag):
                """st_bc [nparts,2]=(sum,sumsq) replicated -> mu,istd."""
                ms = wpool.tile([nparts, 2], F32, tag=tag + "_ms")
                nc.vector.tensor_scalar(ms[:], st_bc[:], 1.0 / n_elems, None,
                                        ALU.mult)
                musq = wpool.tile([nparts, 1], F32, tag=tag + "_musq")
                nc.vector.tensor_mul(musq[:], ms[:, 0:1], ms[:, 0:1])
                var = wpool.tile([nparts, 1], F32, tag=tag + "_var")
                nc.vector.tensor_sub(var[:], ms[:, 1:2], musq[:])
                vare = wpool.tile([nparts, 1], F32, tag=tag + "_vare")
                nc.vector.tensor_scalar(vare[:], var[:], EPS, None, ALU.add)
                lnv = wpool.tile([nparts, 1], F32, tag=tag + "_lnv")
                nc.scalar.activation(lnv[:], vare[:], AOT.Ln)
                istd = wpool.tile([nparts, 1], F32, tag=tag + "_istd")
                nc.scalar.activation(istd[:], lnv[:], AOT.Exp, scale=-0.5)
                return ms[:, 0:1], istd

            mu0_bc, istd0_bc = ln_from_bc(st0_bc, LN0_N, 128, "ln0")

            # ---------- Phase B1: full sem_rs, padded, in SBUF (bf16) ----------
            semrs_pad = []
            for k in range(3):
                sp = wpool.tile([128, 34, 34], BF16, tag=f"semrs_pad{k}")
                nc.vector.memset(sp[:], 0.0)
                for h in range(2):
                    ps = psB.tile([128, 512], F32, tag="big0", bufs=4)
                    nc.tensor.matmul(ps[:],
                                     lhsT=avg_all[:, k * 128:(k + 1) * 128],
                                     rhs=m_all[:, h * 512:(h + 1) * 512],
                                     start=True, stop=True)
                    nc.scalar.activation(
                        sp[:, 1 + h * 16:17 + h * 16, 1:33],
                        ps[:].rearrange("c (r w) -> c r w", r=16), AOT.Copy)
                semrs_pad.append(sp)

            # ---------- shared 3x3 convs (replicated; L=0 now, 1/2 later) ----
            hsh_pad = [wpool.tile([128, 34, 34], BF16, tag=f"hsh_pad{L}",
                                  name=f"hsh_pad{L}")
                       for L in range(3)]

            def build_hsh(L):
                hp = hsh_pad[L]
                nc.vector.memset(hp[:], 0.0)
                for h in range(2):
                    ps = psB.tile([128, 512], F32, tag="big0", bufs=4)
                    first = True
                    for cic in range(3):
                        for t in range(9):
                            dy, dx = t // 3, t % 3
                            nc.tensor.matmul(
                                ps[:].rearrange("c (r w) -> c r w", r=16),
                                lhsT=wsh3_sb[:, L, t, cic, :],
                                rhs=_conv_windows(semrs_pad[cic][:], 16, 32,
                                                  dy, dx, row0=h * 16),
                                start=first, stop=(cic == 2 and t == 8))
                            first = False
                    nc.scalar.activation(
                        hp[:, 1 + h * 16:17 + h * 16, 1:33],
                        ps[:].rearrange("c (r w) -> c r w", r=16), AOT.Relu,
                        bias=bsh3_sb[:, L:L + 1])

            build_hsh(0)

            # ---------- Phase B5/B6: layer-0 gamma/beta convs + SPADE + c0 ----
            c0_ps = [psB.tile([8, 512], F32, tag="acc", bufs=2,
                              name=f"c0_ps{h}") for h in range(2)]
            for m in range(3):
                xn_m = wpool.tile([128, NPIX], BF16, tag="xn", bufs=2)
                nc.vector.tensor_scalar(xn_m[:], x_sb[:, m, :], mu0_bc[:],
                                        istd0_bc[:], ALU.subtract, ALU.mult)
                for h in range(2):
                    gb_ps = {}
                    for name, w_sb in (("g", wg_sb), ("be", wbe_sb)):
                        ps = psB.tile([128, 512], F32,
                                      tag=("big0" if name == "g" else "big1"),
                                      bufs=(4 if name == "g" else 2))
                        for t in range(9):
                            dy, dx = t // 3, t % 3
                            nc.tensor.matmul(
                                ps[:].rearrange("c (r w) -> c r w", r=16),
                                lhsT=w_sb[:, t, m * 128:(m + 1) * 128],
                                rhs=_conv_windows(hsh_pad[0][:], 16, 32, dy, dx,
                                                  row0=h * 16),
                                start=(t == 0), stop=(t == 8))
                        gb_ps[name] = ps
                    # spade: out = xn*(1+gamma+bg) + (beta+bbe)
                    hs = slice(h * 512, (h + 1) * 512)
                    u = wpool.tile([128, 512], BF16, tag="spade_u", bufs=2)
                    nc.vector.scalar_tensor_tensor(u[:], gb_ps["g"][:],
                                                   opg0_sb[:, m:m + 1],
                                                   xn_m[:, hs],
                                                   ALU.add, ALU.mult)
                    sp0 = wpool.tile([128, 512], BF16, tag="spade_o", bufs=2)
                    nc.vector.scalar_tensor_tensor(sp0[:], gb_ps["be"][:],
                                                   bbe0a_sb[:, m:m + 1], u[:],
                                                   ALU.add, ALU.add)
                    nc.tensor.matmul(c0_ps[h][:], lhsT=wc0t_sb[:, m, :],
                                     rhs=sp0[:], start=(m == 0), stop=(m == 2))

            c0p_sb = wpool.tile([8, NPIX], F32, tag="f32buf")
            for h in range(2):
                nc.scalar.activation(c0p_sb[:, h * 512:(h + 1) * 512],
                                     c0_ps[h][:], AOT.Copy)
            nc.sync.dma_start(out=cc0_in[:], in_=c0p_sb[:])
            nc.gpsimd.collective_compute(
                "AllReduce", ALU.add,
                replica_groups=[[0, 1, 2, 3], [4, 5, 6, 7]],
                ins=[cc0_in[:]], outs=[cc0_out[:]],
            )

            # emitted after the collective trigger: overlaps C2 on the PE
            build_hsh(1)
            build_hsh(2)

            # ---------- Phase B7: h1 + LN1 ----------
            def softplus_to(dst, src_ap, bias_sb, nparts, tag):
                """dst = ln(1 + exp(src + bias)); inputs here are small, so
                exp cannot overflow and both ACTs share one table set."""
                e = wpool.tile([nparts, NPIX], F32, tag=tag + "_e")
                nc.scalar.activation(e[:], src_ap, AOT.Exp, bias=bias_sb[:])
                nc.scalar.activation(dst[:], e[:], AOT.Ln, bias=1.0)

            c0_sb = wpool.tile([8, NPIX], F32, tag="f32buf2")
            nc.sync.dma_start(out=c0_sb[:], in_=cc0_out[:])
            h1_f32 = wpool.tile([8, NPIX], F32, tag="hbuf")
            softplus_to(h1_f32, c0_sb[:], b0_sb, 8, "sp1")

            def ln_small(h_f32, nparts, n_elems, tag):
                hsum = wpool.tile([nparts, 1], F32, tag=tag + "_hsum")
                nc.vector.tensor_reduce(hsum[:], h_f32[:], mybir.AxisListType.X,
                                        ALU.add)
                hsq = wpool.tile([nparts, NPIX], BF16, tag="sq_scratch2")
                hsumsq = wpool.tile([nparts, 1], F32, tag=tag + "_hsumsq")
                nc.scalar.activation(hsq[:], h_f32[:], AOT.Square,
                                     accum_out=hsumsq[:])
                st2 = wpool.tile([nparts, 2], F32, tag=tag + "_st2")
                nc.vector.tensor_copy(st2[:, 0:1], hsum[:])
                nc.vector.tensor_copy(st2[:, 1:2], hsumsq[:])
                st_ps = psB.tile([128, 2], F32, tag="acc", bufs=2)
                nc.tensor.matmul(st_ps[:], lhsT=onesbc_sb[0:nparts, :],
                                 rhs=st2[:], start=True, stop=True)
                st_bc = wpool.tile([nparts, 2], F32, tag=tag + "_stbc")
                nc.scalar.activation(st_bc[:], st_ps[0:nparts, :], AOT.Copy)
                return ln_from_bc(st_bc, n_elems, nparts, tag)

            mu1_bc, istd1_bc = ln_small(h1_f32, 8, LN1_N, "ln1")
            xn1 = wpool.tile([8, NPIX], BF16, tag="xn1")
            nc.vector.tensor_scalar(xn1[:], h1_f32[:], mu1_bc[:], istd1_bc[:],
                                    ALU.subtract, ALU.mult)

            # ---------- Phase B8: layers 1 and 2 (replicated) ----------
            def spade_small(xn_bf, nco, wgbe_sb, opg_sb, bbea_sb, tag):
                """wgbe_sb [128, 9, 2*nco]: gamma cols 0:nco, beta nco:2nco —
                one matmul stream produces both."""
                sp_ = wpool.tile([nco, NPIX], BF16, tag=f"{tag}_sp")
                pad_img = hsh_pad[1 if tag == "l1" else 2]
                for h in range(2):
                    p = psB.tile([32 + nco, 512], F32, tag="big0", bufs=4)
                    for t in range(9):
                        dy, dx = t // 3, t % 3
                        nc.tensor.matmul(
                            p[:].rearrange("c (r w) -> c r w", r=16),
                            lhsT=wgbe_sb[:, t, :],
                            rhs=_conv_windows(pad_img[:], 16, 32, dy, dx,
                                              row0=h * 16),
                            start=(t == 0), stop=(t == 8))
                    hs = slice(h * 512, (h + 1) * 512)
                    u_ = wpool.tile([nco, 512], BF16, tag=f"{tag}_u")
                    nc.vector.scalar_tensor_tensor(u_[:], p[0:nco, :],
                                                   opg_sb[:], xn_bf[:, hs],
                                                   ALU.add, ALU.mult)
                    nc.vector.scalar_tensor_tensor(sp_[:, hs], p[32:32 + nco, :],
                                                   bbea_sb[:], u_[:],
                                                   ALU.add, ALU.add)
                return sp_

            sp1 = spade_small(xn1, 8, wgbe1_sb, opg1_sb, bbe1a_sb, "l1")
            c1_sb = wpool.tile([16, NPIX], F32, tag="f32buf")
            for h in range(2):
                c1_ps = psB.tile([16, 512], F32, tag="acc", bufs=2)
                nc.tensor.matmul(c1_ps[:], lhsT=wc1t_sb[:],
                                 rhs=sp1[:, h * 512:(h + 1) * 512],
                                 start=True, stop=True)
                nc.scalar.activation(c1_sb[:, h * 512:(h + 1) * 512], c1_ps[:],
                                     AOT.Copy)
            h2_f32 = wpool.tile([16, NPIX], F32, tag="hbuf2")
            softplus_to(h2_f32, c1_sb[:], b1_sb, 16, "sp2")

            mu2_bc, istd2_bc = ln_small(h2_f32, 16, LN2_N, "ln2")
            xn2 = wpool.tile([16, NPIX], BF16, tag="xn2")
            nc.vector.tensor_scalar(xn2[:], h2_f32[:], mu2_bc[:], istd2_bc[:],
                                    ALU.subtract, ALU.mult)

            sp2 = spade_small(xn2, 16, wgbe2_sb, opg2_sb, bbe2a_sb, "l2")
            c2_sb = wpool.tile([1, NPIX], F32, tag="f32buf3")
            for h in range(2):
                c2_ps = psB.tile([1, 512], F32, tag="acc", bufs=2)
                nc.tensor.matmul(c2_ps[:], lhsT=wc2t_sb[:],
                                 rhs=sp2[:, h * 512:(h + 1) * 512],
                                 start=True, stop=True)
                nc.scalar.activation(c2_sb[:, h * 512:(h + 1) * 512], c2_ps[:],
                                     AOT.Copy)
            out_f32 = wpool.tile([1, NPIX], F32, tag="hbuf3")
            softplus_to(out_f32, c2_sb[:], b2_sb, 1, "sp3")
            nc.sync.dma_start(out=out_t[:], in_=out_f32[:])
            psB_ctx.__exit__(None, None, None)

    nc.finalize()
    _split_sync_waits(nc)
    return nc


def _pack_inputs(inputs):
    f32 = np.float32
    R = _resize_matrix(HI, HP)      # [32, 448]
    C = _resize_matrix(WI, WP)      # [32, 448]
    rt = np.zeros((112, 4, 32), f32)
    ctm = np.zeros((112, 4, 32), f32)
    for c in range(4):
        rt[:, c, :] = R[:, c * 112:(c + 1) * 112].T
        ctm[:, c, :] = C[:, c * 112:(c + 1) * 112].T
    ident = np.eye(32, dtype=f32)

    segmap = inputs["segmap"]            # [2, 448, 448] int32
    f_sem = inputs["f_semantic"]         # [2, 384, 32, 32]
    x_main = inputs["x_main"]            # [2, 1536, 32, 32]
    rows = (np.arange(HP) * HI) // HP

    def tap_t(w):  # [co, ci, 3, 3] -> [ci, 9, co]
        return np.ascontiguousarray(w.transpose(1, 2, 3, 0).reshape(
            w.shape[1], 9, w.shape[0]))

    # all three shared conv layers: [ci_p, L, tap, cic, co]
    wsh3 = np.stack([
        tap_t(inputs[f"w_sh{L}"]).reshape(3, 128, 9, HM).transpose(1, 2, 0, 3)
        for L in range(3)], axis=1).astype(BF16_NP)
    bsh3 = np.stack([inputs[f"b_sh{L}"] for L in range(3)], axis=1).astype(f32)

    maps = []
    for cid in range(8):
        b, g = cid // G, cid % G
        d = {}
        seg = segmap[b].astype(f32)
        d["segbf"] = seg.reshape(4, 112, WI).transpose(1, 0, 2).astype(BF16_NP)
        d["segval"] = np.broadcast_to(
            (np.arange(SEGC, dtype=f32) + SEGC * g)[None, :], (128, SEGC)
        ).astype(f32).copy()
        d["rt"] = rt.astype(BF16_NP)
        d["ct"] = ctm.astype(BF16_NP)
        d["ident"] = ident.astype(BF16_NP)
        seg_small = seg[rows[:, None], rows[None, :]].reshape(-1)   # [1024]
        d["segsm"] = seg_small.reshape(8, 128).T.astype(BF16_NP).copy()
        fT = f_sem[b].reshape(CS, NPIX).T                           # [1024, 384]
        fTe = np.concatenate([fT, np.ones((NPIX, 1), f32)], 1)      # [1024, 385]
        d["fsemT"] = fTe.reshape(8, 128, 385).transpose(1, 0, 2).astype(
            BF16_NP).copy()
        xc = x_main[b, g * COC:(g + 1) * COC].reshape(COC, NPIX)
        d["xq"] = np.ascontiguousarray(
            xc.reshape(3, 128, NPIX).transpose(1, 0, 2))
        d["wsh3"] = wsh3
        d["bsh3"] = bsh3
        cosl = slice(g * COC, (g + 1) * COC)
        d["wg"] = tap_t(inputs["w_g0"][cosl]).astype(BF16_NP)      # [128,9,384]
        d["wbe"] = tap_t(inputs["w_be0"][cosl]).astype(BF16_NP)
        d["opg0"] = np.ascontiguousarray(
            (1.0 + inputs["b_g0"][cosl]).reshape(3, 128).T).astype(f32)
        d["bbe0a"] = np.ascontiguousarray(
            inputs["b_be0"][cosl].reshape(3, 128).T).astype(f32)
        wc0 = inputs["w_c0"][:, :, 0, 0]                           # [8, 1536]
        d["wc0t"] = np.ascontiguousarray(
            wc0[:, cosl].T.reshape(3, 128, 8).transpose(1, 0, 2)).astype(BF16_NP)
        def gbe_pack(wg_, wbe_, nco):   # [128, 9, 32+nco], beta at col 32
            out = np.zeros((128, 9, 32 + nco), f32)
            out[:, :, 0:nco] = tap_t(wg_)
            out[:, :, 32:32 + nco] = tap_t(wbe_)
            return out.astype(BF16_NP)

        d["wgbe1"] = gbe_pack(inputs["w_g1"], inputs["w_be1"], 8)
        d["opg1"] = (1.0 + inputs["b_g1"]).reshape(8, 1).astype(f32)
        d["bbe1a"] = inputs["b_be1"].reshape(8, 1).astype(f32)
        d["wgbe2"] = gbe_pack(inputs["w_g2"], inputs["w_be2"], 16)
        d["opg2"] = (1.0 + inputs["b_g2"]).reshape(16, 1).astype(f32)
        d["bbe2a"] = inputs["b_be2"].reshape(16, 1).astype(f32)
        d["wc1t"] = inputs["w_c1"][:, :, 0, 0].T.astype(BF16_NP).copy()  # [8,16]
        d["wc2t"] = inputs["w_c2"][:, :, 0, 0].T.astype(BF16_NP).copy()  # [16,1]
        d["b0"] = inputs["bias0"].reshape(8, 1).astype(f32)
        d["b1"] = inputs["bias1"].reshape(16, 1).astype(f32)
        d["b2"] = inputs["bias2"].reshape(1, 1).astype(f32)
        d["ones128f"] = np.ones((128, 1), f32)
        d["onesbc"] = np.ones((16, 128), f32)
        maps.append(d)
    return maps


def kernel(**inputs):
    if "nc" not in _NC_CACHE:
        _NC_CACHE["nc"] = _build_nc()
    nc = _NC_CACHE["nc"]
    in_maps = _pack_inputs(inputs)
    res = run_bass_kernel_spmd(nc, in_maps, list(range(8)))
    out = np.zeros((B, 1, HP, WP), np.float32)
    out[0, 0] = res.results[0]["out"].reshape(HP, WP)
    out[1, 0] = res.results[4]["out"].reshape(HP, WP)
    return out


if __name__ == "__main__":
    nc = _build_nc()
    print("built OK; instructions:",
          sum(len(b.instructions) for f in nc.m.functions for b in f.blocks))


# revision 20
# speedup vs baseline: 1.3802x; 1.0329x over previous
"""Trainium2 Bass kernel for nn_DinoGazeSpade (segment_reduce + SPADE stack).

Strategy (8 NeuronCores, SPMD single program):
  - Two groups of 4 cores; group = batch index b (0..1), rank g = core % 4.
  - Painted-map + bilinear resize is reformulated as segment matrices:
        sem_rs[c,p,q] = sum_s avg[s,c] * M[s,p,q],
        M[s] = R @ onehot_s @ C^T   (R, C: 32x448 separable resize matrices)
    Each core builds avg + M for its 16 segments only, then ONE small
    in-group AllGather ships (avg ‖ M ‖ LN0 stats) [17,1408] bf16 (~48KB);
    every core then computes the FULL sem_rs with a cheap k=64 matmul set.
  - The three shared 3x3 convs (w_sh*) are fully replicated (no AllGather):
    sh0 right after sem_rs; sh1/sh2 are emitted after the c0 AllReduce
    trigger so their matmuls overlap the collective.
  - gamma/beta convs of layer 0 are split by output channel (384 per core);
    the pointwise c0 conv partials are AllReduced (C2).  Layers 1-2 are tiny
    and replicated.
  - conv3x3 = 9 shifted matmuls over a zero-padded [C,34,34] SBUF image.
  - LayerNorm scalars use a ones-matmul reduce+broadcast (no DRAM roundtrip)
    and Rsqrt; softplus is the native ACT Softplus.

The host side packs per-core shards / weight transposes (layout only) and
reassembles the [2,1,32,32] output from cores 0 and 4.
"""

import numpy as np

from concourse import bass, tile, mybir
from concourse.bass_utils import run_bass_kernel_spmd

F32 = mybir.dt.float32
BF16 = mybir.dt.bfloat16
BF16_NP = mybir.dt.np(BF16)
AOT = mybir.ActivationFunctionType
ALU = mybir.AluOpType

# Problem dims
B, CM, CS, HP, WP, HI, WI, HM, NSEG = 2, 1536, 384, 32, 32, 448, 448, 128, 64
G = 4              # cores per batch group
SEGC = NSEG // G   # segments per core = 16
COC = CM // G      # gamma/beta out-channel chunk per core = 384
NPIX = HP * WP     # 1024
EPS = 1e-12
LN0_N = float(CM * NPIX)
LN1_N = float(8 * NPIX)
LN2_N = float(16 * NPIX)
PAYW = CS + NPIX   # 1408 payload cols: avg | M

_NC_CACHE = {}


def _resize_matrix(n_in, n_out):
    """Row matrix of jax.image.resize(..., 'bilinear') for downsampling
    (antialiased triangle kernel, normalized rows). Verified vs jax."""
    scale = n_out / n_in
    p = np.arange(n_out, dtype=np.float64)[:, None]
    i = np.arange(n_in, dtype=np.float64)[None, :]
    center = (p + 0.5) / scale - 0.5
    w = np.maximum(0.0, 1.0 - np.abs(i - center) * scale)
    w = w / w.sum(axis=1, keepdims=True)
    return w.astype(np.float32)


def _split_sync_waits(nc, max_waits=1):
    """walrus in this container encodes at most one sync-wait per
    instruction; hoist extras onto preceding same-engine NoOps."""
    n = 0
    for fn in nc.m.functions:
        for blk in fn.blocks:
            new_insts = []
            for inst in blk.instructions:
                si = getattr(inst, "sync_info", None)
                if si is not None and si.on_wait and len(si.on_wait) > max_waits:
                    waits = list(si.on_wait)
                    head, rest = waits[:-max_waits], waits[-max_waits:]
                    for i in range(0, len(head), max_waits):
                        new_insts.append(mybir.InstNoOp(
                            name=f"I-ws-{nc.next_id()}", engine=inst.engine,
                            ins=[], outs=[],
                            sync_info=mybir.SyncInfo(
                                on_wait=list(head[i:i + max_waits]), on_update=[]),
                        ))
                    si.on_wait = rest
                    n += 1
                new_insts.append(inst)
            blk.instructions = new_insts
    return n


def _conv_windows(pad_ap, rows, cols, dy, dx, row0=0):
    """AP view [P, rows, cols] of a padded [P, 34, 34] image at tap (dy,dx)."""
    return pad_ap[:, row0 + dy:row0 + dy + rows, dx:dx + cols]


def _build_nc():
    nc = bass.Bass()

    def inp(name, shape, dtype):
        return nc.declare_dram_parameter(name, list(shape), dtype, isOutput=False)

    # --- inputs (per-core packed shards; see _pack_inputs) ---
    segbf = inp("segbf", [112, 4, 448], BF16)
    segval = inp("segval", [128, SEGC], F32)
    rt = inp("rt", [112, 4, 32], BF16)
    ct = inp("ct", [112, 4, 32], BF16)
    ident = inp("ident", [32, 32], BF16)
    segsm = inp("segsm", [128, 8], BF16)
    fsemT = inp("fsemT", [128, 8, 385], BF16)
    xq = inp("xq", [128, 3, NPIX], F32)
    wsh3 = inp("wsh3", [128, 3, 9, 3, 128], BF16)  # all sh: [ci_p, L, tap, cic, co]
    bsh3 = inp("bsh3", [128, 3], F32)              # all b_sh: [co, L]
    wg = inp("wg", [128, 9, COC], BF16)            # w_g0 chunk:  [ci, tap, co_local]
    wbe = inp("wbe", [128, 9, COC], BF16)
    opg0 = inp("opg0", [128, 3], F32)      # 1 + b_g0 chunk, [ci_p, m]
    bbe0a = inp("bbe0a", [128, 3], F32)    # b_be0 chunk, [ci_p, m]
    wc0t = inp("wc0t", [128, 3, 8], BF16)
    wgbe1 = inp("wgbe1", [128, 9, 40], BF16)   # gamma @0:8, beta @32:40
    opg1 = inp("opg1", [8, 1], F32)
    bbe1a = inp("bbe1a", [8, 1], F32)
    wgbe2 = inp("wgbe2", [128, 9, 48], BF16)   # gamma @0:16, beta @32:48
    opg2 = inp("opg2", [16, 1], F32)
    bbe2a = inp("bbe2a", [16, 1], F32)
    wc1t = inp("wc1t", [8, 16], BF16)
    wc2t = inp("wc2t", [16, 1], BF16)
    b0 = inp("b0", [8, 1], F32)
    b1 = inp("b1", [16, 1], F32)
    b2 = inp("b2", [1, 1], F32)
    ones128f = inp("ones128f", [128, 1], F32)   # col of ones (reduce lhsT)
    onesbc = inp("onesbc", [16, 128], F32)      # ones block (reduce+broadcast)

    out_t = nc.declare_dram_parameter("out", [1, NPIX], F32, isOutput=True)

    with tile.TileContext(nc) as tc:
        with (
            tc.tile_pool(name="const", bufs=1) as cpool,
            tc.tile_pool(name="work", bufs=1) as wpool,
            tc.tile_pool(name="seg", bufs=3) as segpool,
            tc.tile_pool(name="dram", bufs=1, space="DRAM") as dpool,
        ):
            # ---------- load constants / inputs into SBUF ----------
            def load(pool, ap, dtype=None, name=None):
                t = pool.tile(list(ap.shape), dtype or ap.dtype, tag=name)
                nc.sync.dma_start(out=t[:], in_=ap[:])
                return t

            seg_sb = load(cpool, segbf, name="seg_sb")
            segval_sb = load(cpool, segval, name="segval_sb")
            rt_sb = load(cpool, rt, name="rt_sb")
            ct_sb = load(cpool, ct, name="ct_sb")
            ident_sb = load(cpool, ident, name="ident_sb")
            segsm_sb = load(cpool, segsm, name="segsm_sb")
            fsemT_sb = load(cpool, fsemT, name="fsemT_sb")
            x_sb = load(cpool, xq, name="x_sb")
            ones128f_sb = load(cpool, ones128f, name="ones128f_sb")
            onesbc_sb = load(cpool, onesbc, name="onesbc_sb")
            wsh3_sb = load(cpool, wsh3, name="wsh3_sb")
            bsh3_sb = load(cpool, bsh3, name="bsh3_sb")
            wg_sb = load(cpool, wg, name="wg_sb")
            wbe_sb = load(cpool, wbe, name="wbe_sb")
            opg0_sb = load(cpool, opg0, name="opg0_sb")
            bbe0a_sb = load(cpool, bbe0a, name="bbe0a_sb")
            wc0t_sb = load(cpool, wc0t, name="wc0t_sb")
            wgbe1_sb = load(cpool, wgbe1, name="wgbe1_sb")
            opg1_sb = load(cpool, opg1, name="opg1_sb")
            bbe1a_sb = load(cpool, bbe1a, name="bbe1a_sb")
            wgbe2_sb = load(cpool, wgbe2, name="wgbe2_sb")
            opg2_sb = load(cpool, opg2, name="opg2_sb")
            bbe2a_sb = load(cpool, bbe2a, name="bbe2a_sb")
            wc1t_sb = load(cpool, wc1t, name="wc1t_sb")
            wc2t_sb = load(cpool, wc2t, name="wc2t_sb")
            b0_sb = load(cpool, b0, name="b0_sb")
            b1_sb = load(cpool, b1, name="b1_sb")
            b2_sb = load(cpool, b2, name="b2_sb")

            # DRAM scratch
            crs_in = dpool.tile([SEGC + 1, PAYW], BF16)
            crs_out = dpool.tile([G, SEGC + 1, PAYW], BF16)
            cc0_in = dpool.tile([8, NPIX], F32)
            cc0_out = dpool.tile([8, NPIX], F32)

            # ---------- Phase A1: LayerNorm0 partial stats from x chunk ----------
            xsum = wpool.tile([128, 1], F32, tag="xsum")
            nc.vector.tensor_reduce(xsum[:], x_sb[:], mybir.AxisListType.XY, ALU.add)
            xsq_scratch = wpool.tile([128, 3, NPIX], BF16, tag="sq_scratch")
            xsumsq = wpool.tile([128, 1], F32, tag="xsumsq")
            nc.scalar.activation(xsq_scratch[:], x_sb[:], AOT.Square,
                                 accum_out=xsumsq[:])
            stats2 = wpool.tile([128, 2], F32, tag="stats2")
            nc.vector.tensor_copy(stats2[:, 0:1], xsum[:])
            nc.vector.tensor_copy(stats2[:, 1:2], xsumsq[:])

            psA_ctx = tc.tile_pool(name="psA", bufs=1, space="PSUM")
            psA = psA_ctx.__enter__()

            # cross-partition reduce of LN0 partial stats via ones-matmul
            stats1_ps = psA.tile([1, 2], F32, tag="stx")
            nc.tensor.matmul(stats1_ps[:], lhsT=ones128f_sb[:], rhs=stats2[:],
                             start=True, stop=True)
            stats1_sb = wpool.tile([1, 2], F32, tag="stats1_sb")
            nc.scalar.activation(stats1_sb[:], stats1_ps[:], AOT.Copy)
            # ride the AllGather payload as raw bits (bf16 view of f32)
            nc.sync.dma_start(out=crs_in[SEGC:SEGC + 1, 0:4],
                              in_=stats1_sb[:].bitcast(BF16))

            # ---------- Phase A2: segment averages for my 16 segments ----------
            ohsm = wpool.tile([128, SEGC, 8], BF16, tag="ohsm")
            for s in range(SEGC):
                nc.vector.tensor_scalar(ohsm[:, s, :], segsm_sb[:],
                                        segval_sb[:, s:s + 1], None, ALU.is_equal)
            sums_ps = psA.tile([SEGC, 385], F32, tag="sums")
            for c in range(8):
                nc.tensor.matmul(sums_ps[:], lhsT=ohsm[:, :, c],
                                 rhs=fsemT_sb[:, c, :],
                                 start=(c == 0), stop=(c == 7))
            sums_sb = wpool.tile([SEGC, 385], F32, tag="sums_sb")
            nc.scalar.activation(sums_sb[:], sums_ps[:], AOT.Copy)
            cnt_safe = wpool.tile([SEGC, 1], F32, tag="cnt_safe")
            nc.vector.tensor_scalar(cnt_safe[:], sums_sb[:, 384:385], 1.0, None,
                                    ALU.max)
            rec = wpool.tile([SEGC, 1], F32, tag="rec")
            nc.vector.reciprocal(rec[:], cnt_safe[:])
            mask = wpool.tile([SEGC, 1], F32, tag="mask")
            nc.vector.tensor_scalar(mask[:], sums_sb[:, 384:385], 0.5, None,
                                    ALU.is_gt)
            recm = wpool.tile([SEGC, 1], F32, tag="recm")
            nc.vector.tensor_mul(recm[:], rec[:], mask[:])
            avg_bf = wpool.tile([SEGC, CS], BF16, tag="avg_bf")
            nc.vector.tensor_scalar(avg_bf[:], sums_sb[:, 0:384], recm[:], None,
                                    ALU.mult)
            nc.sync.dma_start(out=crs_in[0:SEGC, 0:CS], in_=avg_bf[:])

            # ---------- Phase A3: M matrices for my 16 segments ----------
            mall_ps = psA.tile([32, SEGC * 32], F32, tag="mall")
            for s in range(SEGC):
                oh = segpool.tile([112, 4, 448], BF16, tag="oh")
                nc.vector.tensor_scalar(oh[:], seg_sb[:],
                                        segval_sb[0:112, s:s + 1], None,
                                        ALU.is_equal)
                a_ps = psA.tile([32, 448], F32, tag="aps", bufs=2)
                for c in range(4):
                    nc.tensor.matmul(a_ps[:], lhsT=rt_sb[:, c, :],
                                     rhs=oh[:, c, :],
                                     start=(c == 0), stop=(c == 3))
                a_sb = segpool.tile([32, 448], BF16, tag="asb")
                nc.scalar.activation(a_sb[:], a_ps[:], AOT.Copy)
                at_ps = psA.tile([112, 4, 32], BF16, tag="atps", bufs=2)
                for c in range(4):
                    nc.tensor.transpose(at_ps[:, c, :],
                                        a_sb[:, c * 112:(c + 1) * 112],
                                        ident_sb[:])
                at_sb = segpool.tile([112, 4, 32], BF16, tag="atsb")
                nc.scalar.activation(at_sb[:], at_ps[:], AOT.Copy)
                for c in range(4):
                    nc.tensor.matmul(mall_ps[:, s * 32:(s + 1) * 32],
                                     lhsT=at_sb[:, c, :], rhs=ct_sb[:, c, :],
                                     start=(c == 0), stop=(c == 3))

            # M [32(p), 16(s), 32(q)] -> payload rows [16(s), p*32+q]
            mall_bf = wpool.tile([32, SEGC, 32], BF16, tag="mall_bf")
            nc.vector.tensor_copy(
                mall_bf[:], mall_ps[:].rearrange("p (s q) -> p s q", s=SEGC))
            nc.sync.dma_start(
                out=crs_in[0:SEGC, CS:PAYW].rearrange(
                    "s (p q) -> p s q", p=32),
                in_=mall_bf[:])

            psA_ctx.__exit__(None, None, None)
            psB_ctx = tc.tile_pool(name="psB", bufs=1, space="PSUM")
            psB = psB_ctx.__enter__()

            # ---------- C1: ONE small AllGather of (avg | M | stats) ----------
            nc.gpsimd.collective_compute(
                "AllGather", ALU.bypass,
                replica_groups=[[0, 1, 2, 3], [4, 5, 6, 7]],
                ins=[crs_in[:]], outs=[crs_out[:]],
            )

            # PE filler during the collective: the HAM clock gate re-throttles
            # the PE to 1.2 GHz after ~3.4us idle, and the whole post-gather
            # conv stack would then run cold. Dummy matmuls (never read) keep
            # the activity window busy; they occupy the otherwise-idle PE.
            for i in range(44):
                fl = psB.tile([128, 512], F32, tag="big1", bufs=2,
                              name=f"fill{i}")
                nc.tensor.matmul(fl[:], lhsT=wsh3_sb[:, 0, 0, 0, :],
                                 rhs=xsq_scratch[:, 0, 0:512],
                                 start=True, stop=True)

            # ---------- Phase B0: unpack gather + LN0 scalars ----------
            avg_all = wpool.tile([NSEG, CS], BF16, tag="avg_all")
            m_all = wpool.tile([NSEG, NPIX], BF16, tag="m_all")
            for g in range(G):
                nc.sync.dma_start(out=avg_all[g * SEGC:(g + 1) * SEGC, :],
                                  in_=crs_out[g, 0:SEGC, 0:CS])
                nc.sync.dma_start(out=m_all[g * SEGC:(g + 1) * SEGC, :],
                                  in_=crs_out[g, 0:SEGC, CS:PAYW])
            stats4 = wpool.tile([G, 4], BF16, tag="stats4")
            nc.sync.dma_start(out=stats4[:], in_=crs_out[:, SEGC, 0:4])

            # reduce the 4 partial stats AND broadcast to 128 partitions in
            # one ones-matmul
            st0_ps = psB.tile([128, 2], F32, tag="acc", bufs=2)
            nc.tensor.matmul(st0_ps[:], lhsT=onesbc_sb[0:G, :],
                             rhs=stats4[:].bitcast(F32), start=True, stop=True)
            st0_bc = wpool.tile([128, 2], F32, tag="st0_bc")
            nc.scalar.activation(st0_bc[:], st0_ps[:], AOT.Copy)

            def ln_from_bc(st_bc, n_elems, nparts, tag):
                """st_bc [nparts,2]=(sum,sumsq) replicated -> mu,istd."""
                ms = wpool.tile([nparts, 2], F32, tag=tag + "_ms")
                nc.vector.tensor_scalar(ms[:], st_bc[:], 1.0 / n_elems, None,
                                        ALU.mult)
                musq = wpool.tile([nparts, 1], F32, tag=tag + "_musq")
                nc.vector.tensor_mul(musq[:], ms[:, 0:1], ms[:, 0:1])
                var = wpool.tile([nparts, 1], F32, tag=tag + "_var")
                nc.vector.tensor_sub(var[:], ms[:, 1:2], musq[:])
                vare = wpool.tile([nparts, 1], F32, tag=tag + "_vare")
                nc.vector.tensor_scalar(vare[:], var[:], EPS, None, ALU.add)
                lnv = wpool.tile([nparts, 1], F32, tag=tag + "_lnv")
                nc.scalar.activation(lnv[:], vare[:], AOT.Ln)
                istd = wpool.tile([nparts, 1], F32, tag=tag + "_istd")
                nc.scalar.activation(istd[:], lnv[:], AOT.Exp, scale=-0.5)
                return ms[:, 0:1], istd

            mu0_bc, istd0_bc = ln_from_bc(st0_bc, LN0_N, 128, "ln0")

            # ---------- Phase B1: full sem_rs, padded, in SBUF (bf16) ----------
            semrs_pad = []
            for k in range(3):
                sp = wpool.tile([128, 34, 34], BF16, tag=f"semrs_pad{k}")
                nc.vector.memset(sp[:], 0.0)
                for h in range(2):
                    ps = psB.tile([128, 512], F32, tag="big0", bufs=4)
                    nc.tensor.matmul(ps[:],
                                     lhsT=avg_all[:, k * 128:(k + 1) * 128],
                                     rhs=m_all[:, h * 512:(h + 1) * 512],
                                     start=True, stop=True)
                    nc.scalar.activation(
                        sp[:, 1 + h * 16:17 + h * 16, 1:33],
                        ps[:].rearrange("c (r w) -> c r w", r=16), AOT.Copy)
                semrs_pad.append(sp)

            # ---------- shared 3x3 convs (replicated; L=0 now, 1/2 later) ----
            hsh_pad = [wpool.tile([128, 34, 34], BF16, tag=f"hsh_pad{L}",
                                  name=f"hsh_pad{L}")
                       for L in range(3)]

            def build_hsh(L):
                hp = hsh_pad[L]
                nc.vector.memset(hp[:], 0.0)
                for h in range(2):
                    ps = psB.tile([128, 512], F32, tag="big0", bufs=4)
                    first = True
                    for cic in range(3):
                        for t in range(9):
                            dy, dx = t // 3, t % 3
                            nc.tensor.matmul(
                                ps[:].rearrange("c (r w) -> c r w", r=16),
                                lhsT=wsh3_sb[:, L, t, cic, :],
                                rhs=_conv_windows(semrs_pad[cic][:], 16, 32,
                                                  dy, dx, row0=h * 16),
                                start=first, stop=(cic == 2 and t == 8))
                            first = False
                    nc.scalar.activation(
                        hp[:, 1 + h * 16:17 + h * 16, 1:33],
                        ps[:].rearrange("c (r w) -> c r w", r=16), AOT.Relu,
                        bias=bsh3_sb[:, L:L + 1])

            build_hsh(0)

            # ---------- Phase B5/B6: layer-0 gamma/beta convs + SPADE + c0 ----
            c0_ps = [psB.tile([8, 512], F32, tag="acc", bufs=2,
                              name=f"c0_ps{h}") for h in range(2)]
            for m in range(3):
                xn_m = wpool.tile([128, NPIX], BF16, tag="xn", bufs=2)
                nc.vector.tensor_scalar(xn_m[:], x_sb[:, m, :], mu0_bc[:],
                                        istd0_bc[:], ALU.subtract, ALU.mult)
                for h in range(2):
                    gb_ps = {}
                    for name, w_sb in (("g", wg_sb), ("be", wbe_sb)):
                        ps = psB.tile([128, 512], F32,
                                      tag=("big0" if name == "g" else "big1"),
                                      bufs=(4 if name == "g" else 2))
                        for t in range(9):
                            dy, dx = t // 3, t % 3
                            nc.tensor.matmul(
                                ps[:].rearrange("c (r w) -> c r w", r=16),
                                lhsT=w_sb[:, t, m * 128:(m + 1) * 128],
                                rhs=_conv_windows(hsh_pad[0][:], 16, 32, dy, dx,
                                                  row0=h * 16),
                                start=(t == 0), stop=(t == 8))
                        gb_ps[name] = ps
                    # spade: out = xn*(1+gamma+bg) + (beta+bbe)
                    hs = slice(h * 512, (h + 1) * 512)
                    u = wpool.tile([128, 512], BF16, tag="spade_u", bufs=2)
                    nc.vector.scalar_tensor_tensor(u[:], gb_ps["g"][:],
                                                   opg0_sb[:, m:m + 1],
                                                   xn_m[:, hs],
                                                   ALU.add, ALU.mult)
                    sp0 = wpool.tile([128, 512], BF16, tag="spade_o", bufs=2)
                    nc.vector.scalar_tensor_tensor(sp0[:], gb_ps["be"][:],
                                                   bbe0a_sb[:, m:m + 1], u[:],
                                                   ALU.add, ALU.add)
                    nc.tensor.matmul(c0_ps[h][:], lhsT=wc0t_sb[:, m, :],
                                     rhs=sp0[:], start=(m == 0), stop=(m == 2))

            c0p_sb = wpool.tile([8, NPIX], F32, tag="f32buf")
            for h in range(2):
                nc.scalar.activation(c0p_sb[:, h * 512:(h + 1) * 512],
                                     c0_ps[h][:], AOT.Copy)
            nc.sync.dma_start(out=cc0_in[:], in_=c0p_sb[:])
            nc.gpsimd.collective_compute(
                "AllReduce", ALU.add,
                replica_groups=[[0, 1, 2, 3], [4, 5, 6, 7]],
                ins=[cc0_in[:]], outs=[cc0_out[:]],
            )

            # emitted after the collective trigger: overlaps C2 on the PE
            build_hsh(1)
            build_hsh(2)

            # ---------- Phase B7: h1 + LN1 ----------
            def softplus_to(dst, src_ap, bias_sb, nparts, tag):
                """dst = ln(1 + exp(src + bias)); inputs here are small, so
                exp cannot overflow and both ACTs share one table set."""
                e = wpool.tile([nparts, NPIX], F32, tag=tag + "_e")
                nc.scalar.activation(e[:], src_ap, AOT.Exp, bias=bias_sb[:])
                nc.scalar.activation(dst[:], e[:], AOT.Ln, bias=1.0)

            c0_sb = wpool.tile([8, NPIX], F32, tag="f32buf2")
            nc.sync.dma_start(out=c0_sb[:], in_=cc0_out[:])
            h1_f32 = wpool.tile([8, NPIX], F32, tag="hbuf")
            softplus_to(h1_f32, c0_sb[:], b0_sb, 8, "sp1")

            def ln_small(h_f32, nparts, n_elems, tag):
                hsum = wpool.tile([nparts, 1], F32, tag=tag + "_hsum")
                nc.vector.tensor_reduce(hsum[:], h_f32[:], mybir.AxisListType.X,
                                        ALU.add)
                hsq = wpool.tile([nparts, NPIX], BF16, tag="sq_scratch2")
                hsumsq = wpool.tile([nparts, 1], F32, tag=tag + "_hsumsq")
                nc.scalar.activation(hsq[:], h_f32[:], AOT.Square,
                                     accum_out=hsumsq[:])
                st2 = wpool.tile([nparts, 2], F32, tag=tag + "_st2")
                nc.vector.tensor_copy(st2[:, 0:1], hsum[:])
                nc.vector.tensor_copy(st2[:, 1:2], hsumsq[:])
                st_ps = psB.tile([128, 2], F32, tag="acc", bufs=2)
                nc.tensor.matmul(st_ps[:], lhsT=onesbc_sb[0:nparts, :],
                                 rhs=st2[:], start=True, stop=True)
                st_bc = wpool.tile([nparts, 2], F32, tag=tag + "_stbc")
                nc.scalar.activation(st_bc[:], st_ps[0:nparts, :], AOT.Copy)
                return ln_from_bc(st_bc, n_elems, nparts, tag)

            mu1_bc, istd1_bc = ln_small(h1_f32, 8, LN1_N, "ln1")
            xn1 = wpool.tile([8, NPIX], BF16, tag="xn1")
            nc.vector.tensor_scalar(xn1[:], h1_f32[:], mu1_bc[:], istd1_bc[:],
                                    ALU.subtract, ALU.mult)

            # ---------- Phase B8: layers 1 and 2 (replicated) ----------
            def spade_small(xn_bf, nco, wgbe_sb, opg_sb, bbea_sb, tag):
                """wgbe_sb [128, 9, 2*nco]: gamma cols 0:nco, beta nco:2nco —
                one matmul stream produces both."""
                sp_ = wpool.tile([nco, NPIX], BF16, tag=f"{tag}_sp")
                pad_img = hsh_pad[1 if tag == "l1" else 2]
                for h in range(2):
                    p = psB.tile([32 + nco, 512], F32, tag="big0", bufs=4)
                    for t in range(9):
                        dy, dx = t // 3, t % 3
                        nc.tensor.matmul(
                            p[:].rearrange("c (r w) -> c r w", r=16),
                            lhsT=wgbe_sb[:, t, :],
                            rhs=_conv_windows(pad_img[:], 16, 32, dy, dx,
                                              row0=h * 16),
                            start=(t == 0), stop=(t == 8))
                    hs = slice(h * 512, (h + 1) * 512)
                    u_ = wpool.tile([nco, 512], BF16, tag=f"{tag}_u")
                    nc.vector.scalar_tensor_tensor(u_[:], p[0:nco, :],
                                                   opg_sb[:], xn_bf[:, hs],
                                                   ALU.add, ALU.mult)
                    nc.vector.scalar_tensor_tensor(sp_[:, hs], p[32:32 + nco, :],
                                                   bbea_sb[:], u_[:],
                                                   ALU.add, ALU.add)
                return sp_

            sp1 = spade_small(xn1, 8, wgbe1_sb, opg1_sb, bbe1a_sb, "l1")
            c1_sb = wpool.tile([16, NPIX], F32, tag="f32buf")
            for h in range(2):
                c1_ps = psB.tile([16, 512], F32, tag="acc", bufs=2)
                nc.tensor.matmul(c1_ps[:], lhsT=wc1t_sb[:],
                                 rhs=sp1[:, h * 512:(h + 1) * 512],
                                 start=True, stop=True)
                nc.scalar.activation(c1_sb[:, h * 512:(h + 1) * 512], c1_ps[:],
                                     AOT.Copy)
            h2_f32 = wpool.tile([16, NPIX], F32, tag="hbuf2")
            softplus_to(h2_f32, c1_sb[:], b1_sb, 16, "sp2")

            mu2_bc, istd2_bc = ln_small(h2_f32, 16, LN2_N, "ln2")
            xn2 = wpool.tile([16, NPIX], BF16, tag="xn2")
            nc.vector.tensor_scalar(xn2[:], h2_f32[:], mu2_bc[:], istd2_bc[:],
                                    ALU.subtract, ALU.mult)

            sp2 = spade_small(xn2, 16, wgbe2_sb, opg2_sb, bbe2a_sb, "l2")
            c2_sb = wpool.tile([1, NPIX], F32, tag="f32buf3")
            for h in range(2):
                c2_ps = psB.tile([1, 512], F32, tag="acc", bufs=2)
                nc.tensor.matmul(c2_ps[:], lhsT=wc2t_sb[:],
                                 rhs=sp2[:, h * 512:(h + 1) * 512],
                                 start=True, stop=True)
                nc.scalar.activation(c2_sb[:, h * 512:(h + 1) * 512], c2_ps[:],
                                     AOT.Copy)
            out_f32 = wpool.tile([1, NPIX], F32, tag="hbuf3")
            softplus_to(out_f32, c2_sb[:], b2_sb, 1, "sp3")
            nc.sync.dma_start(out=out_t[:], in_=out_f32[:])
            psB_ctx.__exit__(None, None, None)

    nc.finalize()
    _split_sync_waits(nc)
    return nc


def _pack_inputs(inputs):
    f32 = np.float32
    R = _resize_matrix(HI, HP)      # [32, 448]
    C = _resize_matrix(WI, WP)      # [32, 448]
    rt = np.zeros((112, 4, 32), f32)
    ctm = np.zeros((112, 4, 32), f32)
    for c in range(4):
        rt[:, c, :] = R[:, c * 112:(c + 1) * 112].T
        ctm[:, c, :] = C[:, c * 112:(c + 1) * 112].T
    ident = np.eye(32, dtype=f32)

    segmap = inputs["segmap"]            # [2, 448, 448] int32
    f_sem = inputs["f_semantic"]         # [2, 384, 32, 32]
    x_main = inputs["x_main"]            # [2, 1536, 32, 32]
    rows = (np.arange(HP) * HI) // HP

    def tap_t(w):  # [co, ci, 3, 3] -> [ci, 9, co]
        return np.ascontiguousarray(w.transpose(1, 2, 3, 0).reshape(
            w.shape[1], 9, w.shape[0]))

    # all three shared conv layers: [ci_p, L, tap, cic, co]
    wsh3 = np.stack([
        tap_t(inputs[f"w_sh{L}"]).reshape(3, 128, 9, HM).transpose(1, 2, 0, 3)
        for L in range(3)], axis=1).astype(BF16_NP)
    bsh3 = np.stack([inputs[f"b_sh{L}"] for L in range(3)], axis=1).astype(f32)

    maps = []
    for cid in range(8):
        b, g = cid // G, cid % G
        d = {}
        seg = segmap[b].astype(f32)
        d["segbf"] = seg.reshape(4, 112, WI).transpose(1, 0, 2).astype(BF16_NP)
        d["segval"] = np.broadcast_to(
            (np.arange(SEGC, dtype=f32) + SEGC * g)[None, :], (128, SEGC)
        ).astype(f32).copy()
        d["rt"] = rt.astype(BF16_NP)
        d["ct"] = ctm.astype(BF16_NP)
        d["ident"] = ident.astype(BF16_NP)
        seg_small = seg[rows[:, None], rows[None, :]].reshape(-1)   # [1024]
        d["segsm"] = seg_small.reshape(8, 128).T.astype(BF16_NP).copy()
        fT = f_sem[b].reshape(CS, NPIX).T                           # [1024, 384]
        fTe = np.concatenate([fT, np.ones((NPIX, 1), f32)], 1)      # [1024, 385]
        d["fsemT"] = fTe.reshape(8, 128, 385).transpose(1, 0, 2).astype(
            BF16_NP).copy()
        xc = x_main[b, g * COC:(g + 1) * COC].reshape(COC, NPIX)
        d["xq"] = np.ascontiguousarray(
            xc.reshape(3, 128, NPIX).transpose(1, 0, 2))
        d["wsh3"] = wsh3
        d["bsh3"] = bsh3
        cosl = slice(g * COC, (g + 1) * COC)
        d["wg"] = tap_t(inputs["w_g0"][cosl]).astype(BF16_NP)      # [128,9,384]
        d["wbe"] = tap_t(inputs["w_be0"][cosl]).astype(BF16_NP)
        d["opg0"] = np.ascontiguousarray(
            (1.0 + inputs["b_g0"][cosl]).reshape(3, 128).T).astype(f32)
        d["bbe0a"] = np.ascontiguousarray(
            inputs["b_be0"][cosl].reshape(3, 128).T).astype(f32)
        wc0 = inputs["w_c0"][:, :, 0, 0]                           # [8, 1536]
        d["wc0t"] = np.ascontiguousarray(
            wc0[:, cosl].T.reshape(3, 128, 8).transpose(1, 0, 2)).astype(BF16_NP)
        def gbe_pack(wg_, wbe_, nco):   # [128, 9, 32+nco], beta at col 32
            out = np.zeros((128, 9, 32 + nco), f32)
            out[:, :, 0:nco] = tap_t(wg_)
            out[:, :, 32:32 + nco] = tap_t(wbe_)
            return out.astype(BF16_NP)

        d["wgbe1"] = gbe_pack(inputs["w_g1"], inputs["w_be1"], 8)
        d["opg1"] = (1.0 + inputs["b_g1"]).reshape(8, 1).astype(f32)
        d["bbe1a"] = inputs["b_be1"].reshape(8, 1).astype(f32)
        d["wgbe2"] = gbe_pack(inputs["w_g2"], inputs["w_be2"], 16)
        d["opg2"] = (1.0 + inputs["b_g2"]).reshape(16, 1).astype(f32)
        d["bbe2a"] = inputs["b_be2"].reshape(16, 1).astype(f32)
        d["wc1t"] = inputs["w_c1"][:, :, 0, 0].T.astype(BF16_NP).copy()  # [8,16]
        d["wc2t"] = inputs["w_c2"][:, :, 0, 0].T.astype(BF16_NP).copy()  # [16,1]
        d["b0"] = inputs["bias0"].reshape(8, 1).astype(f32)
        d["b1"] = inputs["bias1"].reshape(16, 1).astype(f32)
        d["b2"] = inputs["bias2"].reshape(1, 1).astype(f32)
        d["ones128f"] = np.ones((128, 1), f32)
        d["onesbc"] = np.ones((16, 128), f32)
        maps.append(d)
    return maps


def kernel(**inputs):
    if "nc" not in _NC_CACHE:
        _NC_CACHE["nc"] = _build_nc()
    nc = _NC_CACHE["nc"]
    in_maps = _pack_inputs(inputs)
    res = run_bass_kernel_spmd(nc, in_maps, list(range(8)))
    out = np.zeros((B, 1, HP, WP), np.float32)
    out[0, 0] = res.results[0]["out"].reshape(HP, WP)
    out[1, 0] = res.results[4]["out"].reshape(HP, WP)
    return out


if __name__ == "__main__":
    nc = _build_nc()
    print("built OK; instructions:",
          sum(len(b.instructions) for f in nc.m.functions for b in f.blocks))


# revision 30
# speedup vs baseline: 1.4082x; 1.0203x over previous
"""Trainium2 Bass kernel for nn_DinoGazeSpade (segment_reduce + SPADE stack).

Strategy (8 NeuronCores, SPMD single program):
  - Two groups of 4 cores; group = batch index b (0..1), rank g = core % 4.
  - Painted-map + bilinear resize is reformulated as segment matrices:
        sem_rs[c,p,q] = sum_s avg[s,c] * M[s,p,q],
        M[s] = R @ onehot_s @ C^T   (R, C: 32x448 separable resize matrices)
    Each core builds avg + M for its 16 segments only, then ONE small
    in-group AllGather ships (avg ‖ M ‖ LN0 stats) [17,1408] bf16 (~48KB);
    every core then computes the FULL sem_rs with a cheap k=64 matmul set.
  - The three shared 3x3 convs (w_sh*) are fully replicated (no AllGather):
    sh0 right after sem_rs; sh1/sh2 are emitted after the c0 AllReduce
    trigger so their matmuls overlap the collective.
  - gamma/beta convs of layer 0 are split by output channel (384 per core);
    the pointwise c0 conv partials are AllReduced (C2).  Layers 1-2 are tiny
    and replicated.
  - conv3x3 = 9 shifted matmuls over a zero-padded [C,34,34] SBUF image.
  - LayerNorm scalars use a ones-matmul reduce+broadcast (no DRAM roundtrip)
    and Rsqrt; softplus is the native ACT Softplus.

The host side packs per-core shards / weight transposes (layout only) and
reassembles the [2,1,32,32] output from cores 0 and 4.
"""

import numpy as np

from concourse import bass, tile, mybir
from concourse.bass_utils import run_bass_kernel_spmd

F32 = mybir.dt.float32
BF16 = mybir.dt.bfloat16
BF16_NP = mybir.dt.np(BF16)
AOT = mybir.ActivationFunctionType
ALU = mybir.AluOpType

# Problem dims
B, CM, CS, HP, WP, HI, WI, HM, NSEG = 2, 1536, 384, 32, 32, 448, 448, 128, 64
G = 4              # cores per batch group
SEGC = NSEG // G   # segments per core = 16
COC = CM // G      # gamma/beta out-channel chunk per core = 384
NPIX = HP * WP     # 1024
EPS = 1e-12
LN0_N = float(CM * NPIX)
LN1_N = float(8 * NPIX)
LN2_N = float(16 * NPIX)
PAYW = CS + NPIX   # 1408 payload cols: avg | M

_NC_CACHE = {}


def _resize_matrix(n_in, n_out):
    """Row matrix of jax.image.resize(..., 'bilinear') for downsampling
    (antialiased triangle kernel, normalized rows). Verified vs jax."""
    scale = n_out / n_in
    p = np.arange(n_out, dtype=np.float64)[:, None]
    i = np.arange(n_in, dtype=np.float64)[None, :]
    center = (p + 0.5) / scale - 0.5
    w = np.maximum(0.0, 1.0 - np.abs(i - center) * scale)
    w = w / w.sum(axis=1, keepdims=True)
    return w.astype(np.float32)


def _split_sync_waits(nc, max_waits=1):
    """walrus in this container encodes at most one sync-wait per
    instruction; hoist extras onto preceding same-engine NoOps."""
    n = 0
    for fn in nc.m.functions:
        for blk in fn.blocks:
            new_insts = []
            for inst in blk.instructions:
                si = getattr(inst, "sync_info", None)
                if si is not None and si.on_wait and len(si.on_wait) > max_waits:
                    waits = list(si.on_wait)
                    head, rest = waits[:-max_waits], waits[-max_waits:]
                    for i in range(0, len(head), max_waits):
                        new_insts.append(mybir.InstNoOp(
                            name=f"I-ws-{nc.next_id()}", engine=inst.engine,
                            ins=[], outs=[],
                            sync_info=mybir.SyncInfo(
                                on_wait=list(head[i:i + max_waits]), on_update=[]),
                        ))
                    si.on_wait = rest
                    n += 1
                new_insts.append(inst)
            blk.instructions = new_insts
    return n


def _conv_windows(pad_ap, rows, cols, dy, dx, row0=0):
    """AP view [P, rows, cols] of a padded [P, 34, 34] image at tap (dy,dx)."""
    return pad_ap[:, row0 + dy:row0 + dy + rows, dx:dx + cols]


def _build_nc():
    nc = bass.Bass()

    def inp(name, shape, dtype):
        return nc.declare_dram_parameter(name, list(shape), dtype, isOutput=False)

    # --- inputs (per-core packed shards; see _pack_inputs) ---
    segbf = inp("segbf", [112, 4, 448], BF16)
    segval = inp("segval", [128, SEGC], F32)
    rt = inp("rt", [112, 4, 32], BF16)
    ct = inp("ct", [112, 4, 32], BF16)
    ident = inp("ident", [32, 32], BF16)
    segsm = inp("segsm", [128, 8], BF16)
    fsemT = inp("fsemT", [128, 8, 385], BF16)
    xq = inp("xq", [128, 3, NPIX], F32)
    wsh0 = inp("wsh0", [128, 9, 3, 128], BF16)   # shared conv L0 [ci_p,tap,cic,co]
    bsh0 = inp("bsh0", [128, 1], F32)
    wshm = inp("wshm", [128, 9, 3, 128], BF16)   # my rank-pair's L1-or-L2 conv
    bshm = inp("bshm", [128, 1], F32)
    wg = inp("wg", [128, 9, COC], BF16)            # w_g0 chunk:  [ci, tap, co_local]
    wbe = inp("wbe", [128, 9, COC], BF16)
    opg0 = inp("opg0", [128, 3], F32)      # 1 + b_g0 chunk, [ci_p, m]
    bbe0a = inp("bbe0a", [128, 3], F32)    # b_be0 chunk, [ci_p, m]
    wc0t = inp("wc0t", [128, 3, 8], BF16)
    wgbe1 = inp("wgbe1", [128, 9, 40], BF16)   # gamma @0:8, beta @32:40
    opg1 = inp("opg1", [8, 1], F32)
    bbe1a = inp("bbe1a", [8, 1], F32)
    wgbe2 = inp("wgbe2", [128, 9, 48], BF16)   # gamma @0:16, beta @32:48
    opg2 = inp("opg2", [16, 1], F32)
    bbe2a = inp("bbe2a", [16, 1], F32)
    wc1t = inp("wc1t", [8, 16], BF16)
    wc2t = inp("wc2t", [16, 1], BF16)
    b0 = inp("b0", [8, 1], F32)
    b1 = inp("b1", [16, 1], F32)
    b2 = inp("b2", [1, 1], F32)
    ones128f = inp("ones128f", [128, 1], F32)   # col of ones (reduce lhsT)
    onesbc = inp("onesbc", [16, 128], F32)      # ones block (reduce+broadcast)

    out_t = nc.declare_dram_parameter("out", [1, NPIX], F32, isOutput=True)

    with tile.TileContext(nc) as tc:
        with (
            tc.tile_pool(name="const", bufs=1) as cpool,
            tc.tile_pool(name="work", bufs=1) as wpool,
            tc.tile_pool(name="seg", bufs=3) as segpool,
            tc.tile_pool(name="dram", bufs=1, space="DRAM") as dpool,
        ):
            # ---------- load constants / inputs into SBUF ----------
            def load(pool, ap, dtype=None, name=None):
                t = pool.tile(list(ap.shape), dtype or ap.dtype, tag=name)
                nc.sync.dma_start(out=t[:], in_=ap[:])
                return t

            seg_sb = load(cpool, segbf, name="seg_sb")
            segval_sb = load(cpool, segval, name="segval_sb")
            rt_sb = load(cpool, rt, name="rt_sb")
            ct_sb = load(cpool, ct, name="ct_sb")
            ident_sb = load(cpool, ident, name="ident_sb")
            segsm_sb = load(cpool, segsm, name="segsm_sb")
            fsemT_sb = load(cpool, fsemT, name="fsemT_sb")
            x_sb = load(cpool, xq, name="x_sb")
            ones128f_sb = load(cpool, ones128f, name="ones128f_sb")
            onesbc_sb = load(cpool, onesbc, name="onesbc_sb")
            wsh0_sb = load(cpool, wsh0, name="wsh0_sb")
            bsh0_sb = load(cpool, bsh0, name="bsh0_sb")
            wshm_sb = load(cpool, wshm, name="wshm_sb")
            bshm_sb = load(cpool, bshm, name="bshm_sb")
            wg_sb = load(cpool, wg, name="wg_sb")
            wbe_sb = load(cpool, wbe, name="wbe_sb")
            opg0_sb = load(cpool, opg0, name="opg0_sb")
            bbe0a_sb = load(cpool, bbe0a, name="bbe0a_sb")
            wc0t_sb = load(cpool, wc0t, name="wc0t_sb")
            wgbe1_sb = load(cpool, wgbe1, name="wgbe1_sb")
            opg1_sb = load(cpool, opg1, name="opg1_sb")
            bbe1a_sb = load(cpool, bbe1a, name="bbe1a_sb")
            wgbe2_sb = load(cpool, wgbe2, name="wgbe2_sb")
            opg2_sb = load(cpool, opg2, name="opg2_sb")
            bbe2a_sb = load(cpool, bbe2a, name="bbe2a_sb")
            wc1t_sb = load(cpool, wc1t, name="wc1t_sb")
            wc2t_sb = load(cpool, wc2t, name="wc2t_sb")
            b0_sb = load(cpool, b0, name="b0_sb")
            b1_sb = load(cpool, b1, name="b1_sb")
            b2_sb = load(cpool, b2, name="b2_sb")

            # DRAM scratch
            crs_in = dpool.tile([SEGC + 1, PAYW], BF16)
            crs_out = dpool.tile([G, SEGC + 1, PAYW], BF16)
            crs2_in = dpool.tile([128, NPIX], BF16)
            crs2_out = dpool.tile([G, 128, NPIX], BF16)
            cc0_in = dpool.tile([8, NPIX], F32)
            cc0_out = dpool.tile([8, NPIX], F32)

            # ---------- Phase A1: LayerNorm0 partial stats from x chunk ----------
            xsum = wpool.tile([128, 1], F32, tag="xsum")
            nc.vector.tensor_reduce(xsum[:], x_sb[:], mybir.AxisListType.XY, ALU.add)
            xsq_scratch = wpool.tile([128, 3, NPIX], BF16, tag="sq_scratch")
            xsumsq = wpool.tile([128, 1], F32, tag="xsumsq")
            nc.scalar.activation(xsq_scratch[:], x_sb[:], AOT.Square,
                                 accum_out=xsumsq[:])
            stats2 = wpool.tile([128, 2], F32, tag="stats2")
            nc.vector.tensor_copy(stats2[:, 0:1], xsum[:])
            nc.vector.tensor_copy(stats2[:, 1:2], xsumsq[:])

            psA_ctx = tc.tile_pool(name="psA", bufs=1, space="PSUM")
            psA = psA_ctx.__enter__()

            # cross-partition reduce of LN0 partial stats via ones-matmul
            stats1_ps = psA.tile([1, 2], F32, tag="stx")
            nc.tensor.matmul(stats1_ps[:], lhsT=ones128f_sb[:], rhs=stats2[:],
                             start=True, stop=True)
            stats1_sb = wpool.tile([1, 2], F32, tag="stats1_sb")
            nc.scalar.activation(stats1_sb[:], stats1_ps[:], AOT.Copy)
            # ride the AllGather payload as raw bits (bf16 view of f32)
            nc.sync.dma_start(out=crs_in[SEGC:SEGC + 1, 0:4],
                              in_=stats1_sb[:].bitcast(BF16))

            # ---------- Phase A2: segment averages for my 16 segments ----------
            ohsm = wpool.tile([128, SEGC, 8], BF16, tag="ohsm")
            for s in range(SEGC):
                nc.vector.tensor_scalar(ohsm[:, s, :], segsm_sb[:],
                                        segval_sb[:, s:s + 1], None, ALU.is_equal)
            sums_ps = psA.tile([SEGC, 385], F32, tag="sums")
            for c in range(8):
                nc.tensor.matmul(sums_ps[:], lhsT=ohsm[:, :, c],
                                 rhs=fsemT_sb[:, c, :],
                                 start=(c == 0), stop=(c == 7))
            sums_sb = wpool.tile([SEGC, 385], F32, tag="sums_sb")
            nc.scalar.activation(sums_sb[:], sums_ps[:], AOT.Copy)
            cnt_safe = wpool.tile([SEGC, 1], F32, tag="cnt_safe")
            nc.vector.tensor_scalar(cnt_safe[:], sums_sb[:, 384:385], 1.0, None,
                                    ALU.max)
            rec = wpool.tile([SEGC, 1], F32, tag="rec")
            nc.vector.reciprocal(rec[:], cnt_safe[:])
            mask = wpool.tile([SEGC, 1], F32, tag="mask")
            nc.vector.tensor_scalar(mask[:], sums_sb[:, 384:385], 0.5, None,
                                    ALU.is_gt)
            recm = wpool.tile([SEGC, 1], F32, tag="recm")
            nc.vector.tensor_mul(recm[:], rec[:], mask[:])
            avg_bf = wpool.tile([SEGC, CS], BF16, tag="avg_bf")
            nc.vector.tensor_scalar(avg_bf[:], sums_sb[:, 0:384], recm[:], None,
                                    ALU.mult)
            nc.sync.dma_start(out=crs_in[0:SEGC, 0:CS], in_=avg_bf[:])

            # ---------- Phase A3: M matrices for my 16 segments ----------
            mall_ps = psA.tile([32, SEGC * 32], F32, tag="mall")
            for s in range(SEGC):
                oh = segpool.tile([112, 4, 448], BF16, tag="oh")
                nc.vector.tensor_scalar(oh[:], seg_sb[:],
                                        segval_sb[0:112, s:s + 1], None,
                                        ALU.is_equal)
                a_ps = psA.tile([32, 448], F32, tag="aps", bufs=2)
                for c in range(4):
                    nc.tensor.matmul(a_ps[:], lhsT=rt_sb[:, c, :],
                                     rhs=oh[:, c, :],
                                     start=(c == 0), stop=(c == 3))
                a_sb = segpool.tile([32, 448], BF16, tag="asb")
                nc.scalar.activation(a_sb[:], a_ps[:], AOT.Copy)
                at_ps = psA.tile([112, 4, 32], BF16, tag="atps", bufs=2)
                for c in range(4):
                    nc.tensor.transpose(at_ps[:, c, :],
                                        a_sb[:, c * 112:(c + 1) * 112],
                                        ident_sb[:])
                at_sb = segpool.tile([112, 4, 32], BF16, tag="atsb")
                nc.scalar.activation(at_sb[:], at_ps[:], AOT.Copy)
                for c in range(4):
                    nc.tensor.matmul(mall_ps[:, s * 32:(s + 1) * 32],
                                     lhsT=at_sb[:, c, :], rhs=ct_sb[:, c, :],
                                     start=(c == 0), stop=(c == 3))

            # M [32(p), 16(s), 32(q)] -> payload rows [16(s), p*32+q]
            mall_bf = wpool.tile([32, SEGC, 32], BF16, tag="mall_bf")
            nc.vector.tensor_copy(
                mall_bf[:], mall_ps[:].rearrange("p (s q) -> p s q", s=SEGC))
            nc.sync.dma_start(
                out=crs_in[0:SEGC, CS:PAYW].rearrange(
                    "s (p q) -> p s q", p=32),
                in_=mall_bf[:])

            psA_ctx.__exit__(None, None, None)
            psB_ctx = tc.tile_pool(name="psB", bufs=1, space="PSUM")
            psB = psB_ctx.__enter__()

            # ---------- C1: ONE small AllGather of (avg | M | stats) ----------
            nc.gpsimd.collective_compute(
                "AllGather", ALU.bypass,
                replica_groups=[[0, 1, 2, 3], [4, 5, 6, 7]],
                ins=[crs_in[:]], outs=[crs_out[:]],
            )

            # PE filler during the collective: the HAM clock gate re-throttles
            # the PE to 1.2 GHz after ~3.4us idle, and the whole post-gather
            # conv stack would then run cold. Dummy matmuls (never read) keep
            # the activity window busy; they occupy the otherwise-idle PE.
            for i in range(44):
                fl = psB.tile([128, 512], F32, tag="big1", bufs=2,
                              name=f"fill{i}")
                nc.tensor.matmul(fl[:], lhsT=wsh0_sb[:, 0, 0, :],
                                 rhs=xsq_scratch[:, 0, 0:512],
                                 start=True, stop=True)

            # ---------- Phase B0: unpack gather + LN0 scalars ----------
            avg_all = wpool.tile([NSEG, CS], BF16, tag="avg_all")
            m_all = wpool.tile([NSEG, NPIX], BF16, tag="m_all")
            for g in range(G):
                nc.sync.dma_start(out=avg_all[g * SEGC:(g + 1) * SEGC, :],
                                  in_=crs_out[g, 0:SEGC, 0:CS])
                nc.sync.dma_start(out=m_all[g * SEGC:(g + 1) * SEGC, :],
                                  in_=crs_out[g, 0:SEGC, CS:PAYW])
            stats4 = wpool.tile([G, 4], BF16, tag="stats4")
            nc.sync.dma_start(out=stats4[:], in_=crs_out[:, SEGC, 0:4])

            # reduce the 4 partial stats AND broadcast to 128 partitions in
            # one ones-matmul
            st0_ps = psB.tile([128, 2], F32, tag="acc", bufs=2)
            nc.tensor.matmul(st0_ps[:], lhsT=onesbc_sb[0:G, :],
                             rhs=stats4[:].bitcast(F32), start=True, stop=True)
            st0_bc = wpool.tile([128, 2], F32, tag="st0_bc")
            nc.scalar.activation(st0_bc[:], st0_ps[:], AOT.Copy)

            def ln_from_bc(st_bc, n_elems, nparts, tag):
                """st_bc [nparts,2]=(sum,sumsq) replicated -> mu,istd."""
                ms = wpool.tile([nparts, 2], F32, tag=tag + "_ms")
                nc.vector.tensor_scalar(ms[:], st_bc[:], 1.0 / n_elems, None,
                                        ALU.mult)
                musq = wpool.tile([nparts, 1], F32, tag=tag + "_musq")
                nc.vector.tensor_mul(musq[:], ms[:, 0:1], ms[:, 0:1])
                var = wpool.tile([nparts, 1], F32, tag=tag + "_var")
                nc.vector.tensor_sub(var[:], ms[:, 1:2], musq[:])
                vare = wpool.tile([nparts, 1], F32, tag=tag + "_vare")
                nc.vector.tensor_scalar(vare[:], var[:], EPS, None, ALU.add)
                lnv = wpool.tile([nparts, 1], F32, tag=tag + "_lnv")
                nc.scalar.activation(lnv[:], vare[:], AOT.Ln)
                istd = wpool.tile([nparts, 1], F32, tag=tag + "_istd")
                nc.scalar.activation(istd[:], lnv[:], AOT.Exp, scale=-0.5)
                return ms[:, 0:1], istd

            mu0_bc, istd0_bc = ln_from_bc(st0_bc, LN0_N, 128, "ln0")

            # ---------- Phase B1: full sem_rs, padded, in SBUF (bf16) ----------
            semrs_pad = []
            for k in range(3):
                sp = wpool.tile([128, 34, 34], BF16, tag=f"semrs_pad{k}")
                nc.vector.memset(sp[:], 0.0)
                for h in range(2):
                    ps = psB.tile([128, 512], F32, tag="big0", bufs=4)
                    nc.tensor.matmul(ps[:],
                                     lhsT=avg_all[:, k * 128:(k + 1) * 128],
                                     rhs=m_all[:, h * 512:(h + 1) * 512],
                                     start=True, stop=True)
                    nc.scalar.activation(
                        sp[:, 1 + h * 16:17 + h * 16, 1:33],
                        ps[:].rearrange("c (r w) -> c r w", r=16), AOT.Copy)
                semrs_pad.append(sp)

            # ---------- shared 3x3 convs ----------
            # L0 is computed locally by every rank; ranks 0/1 compute the
            # full L1, ranks 2/3 the full L2, then an AllGather (hidden
            # under the L0 gamma/beta convs) distributes them.
            hsh_pad = [wpool.tile([128, 34, 34], BF16, tag=f"hsh_pad{L}",
                                  name=f"hsh_pad{L}")
                       for L in range(3)]

            def sh_conv_half(ps, w_sb, h):
                first = True
                for cic in range(3):
                    for t in range(9):
                        dy, dx = t // 3, t % 3
                        nc.tensor.matmul(
                            ps[:].rearrange("c (r w) -> c r w", r=16),
                            lhsT=w_sb[:, t, cic, :],
                            rhs=_conv_windows(semrs_pad[cic][:], 16, 32,
                                              dy, dx, row0=h * 16),
                            start=first, stop=(cic == 2 and t == 8))
                        first = False

            # my rank-pair's layer first, so the AllGather can start early
            hshm_flat = wpool.tile([128, NPIX], BF16, tag="hshm_flat")
            for h in range(2):
                ps = psB.tile([128, 512], F32, tag="big0", bufs=4)
                sh_conv_half(ps, wshm_sb, h)
                nc.scalar.activation(hshm_flat[:, h * 512:(h + 1) * 512],
                                     ps[:], AOT.Relu, bias=bshm_sb[:])
            nc.sync.dma_start(out=crs2_in[:], in_=hshm_flat[:])
            nc.gpsimd.collective_compute(
                "AllGather", ALU.bypass,
                replica_groups=[[0, 1, 2, 3], [4, 5, 6, 7]],
                ins=[crs2_in[:]], outs=[crs2_out[:]],
            )

            # L0 locally
            nc.vector.memset(hsh_pad[0][:], 0.0)
            for h in range(2):
                ps = psB.tile([128, 512], F32, tag="big0", bufs=4)
                sh_conv_half(ps, wsh0_sb, h)
                nc.scalar.activation(
                    hsh_pad[0][:, 1 + h * 16:17 + h * 16, 1:33],
                    ps[:].rearrange("c (r w) -> c r w", r=16), AOT.Relu,
                    bias=bsh0_sb[:])

            # ---------- Phase B5/B6: layer-0 gamma/beta convs + SPADE + c0 ----
            c0_ps = [psB.tile([8, 512], F32, tag="acc", bufs=2,
                              name=f"c0_ps{h}") for h in range(2)]
            for m in range(3):
                xn_m = wpool.tile([128, NPIX], BF16, tag="xn", bufs=2)
                nc.vector.tensor_scalar(xn_m[:], x_sb[:, m, :], mu0_bc[:],
                                        istd0_bc[:], ALU.subtract, ALU.mult)
                for h in range(2):
                    gb_ps = {}
                    for name, w_sb in (("g", wg_sb), ("be", wbe_sb)):
                        ps = psB.tile([128, 512], F32,
                                      tag=("big0" if name == "g" else "big1"),
                                      bufs=(4 if name == "g" else 2))
                        for t in range(9):
                            dy, dx = t // 3, t % 3
                            nc.tensor.matmul(
                                ps[:].rearrange("c (r w) -> c r w", r=16),
                                lhsT=w_sb[:, t, m * 128:(m + 1) * 128],
                                rhs=_conv_windows(hsh_pad[0][:], 16, 32, dy, dx,
                                                  row0=h * 16),
                                start=(t == 0), stop=(t == 8))
                        gb_ps[name] = ps
                    # spade: out = xn*(1+gamma+bg) + (beta+bbe)
                    hs = slice(h * 512, (h + 1) * 512)
                    u = wpool.tile([128, 512], BF16, tag="spade_u", bufs=2)
                    nc.vector.scalar_tensor_tensor(u[:], gb_ps["g"][:],
                                                   opg0_sb[:, m:m + 1],
                                                   xn_m[:, hs],
                                                   ALU.add, ALU.mult)
                    sp0 = wpool.tile([128, 512], BF16, tag="spade_o", bufs=2)
                    nc.vector.scalar_tensor_tensor(sp0[:], gb_ps["be"][:],
                                                   bbe0a_sb[:, m:m + 1], u[:],
                                                   ALU.add, ALU.add)
                    nc.tensor.matmul(c0_ps[h][:], lhsT=wc0t_sb[:, m, :],
                                     rhs=sp0[:], start=(m == 0), stop=(m == 2))

            c0p_sb = wpool.tile([8, NPIX], F32, tag="f32buf")
            for h in range(2):
                nc.scalar.activation(c0p_sb[:, h * 512:(h + 1) * 512],
                                     c0_ps[h][:], AOT.Copy)
            nc.sync.dma_start(out=cc0_in[:], in_=c0p_sb[:])
            nc.gpsimd.collective_compute(
                "AllReduce", ALU.add,
                replica_groups=[[0, 1, 2, 3], [4, 5, 6, 7]],
                ins=[cc0_in[:]], outs=[cc0_out[:]],
            )

            # emitted after the collective trigger: unpack gathered sh layers
            # and prefire the L1/L2 conv matmuls so they overlap C2 on the PE
            hshg = [None, None]
            for L in (1, 2):
                hg = wpool.tile([128, NPIX], BF16, tag=f"hshg{L}",
                                name=f"hshg{L}")
                nc.sync.dma_start(out=hg[:], in_=crs2_out[0 if L == 1 else 2])
                hp = hsh_pad[L]
                nc.vector.memset(hp[:], 0.0)
                nc.vector.tensor_copy(
                    hp[:, 1:33, 1:33],
                    hg[:].rearrange("c (r w) -> c r w", r=32))
                hshg[L - 1] = hg

            def spade_conv_ps(nco, wgbe_sb, pad_img):
                pss = []
                for h in range(2):
                    p = psB.tile([32 + nco, 512], F32, tag="big0", bufs=4)
                    for t in range(9):
                        dy, dx = t // 3, t % 3
                        nc.tensor.matmul(
                            p[:].rearrange("c (r w) -> c r w", r=16),
                            lhsT=wgbe_sb[:, t, :],
                            rhs=_conv_windows(pad_img[:], 16, 32, dy, dx,
                                              row0=h * 16),
                            start=(t == 0), stop=(t == 8))
                    pss.append(p)
                return pss

            l1_ps = spade_conv_ps(8, wgbe1_sb, hsh_pad[1])
            l2_ps = spade_conv_ps(16, wgbe2_sb, hsh_pad[2])

            # ---------- Phase B7: h1 + LN1 ----------
            def softplus_to(dst, src_ap, bias_sb, nparts, tag):
                """dst = ln(1 + exp(src + bias)); inputs here are small, so
                exp cannot overflow and both ACTs share one table set."""
                e = wpool.tile([nparts, NPIX], F32, tag=tag + "_e")
                nc.scalar.activation(e[:], src_ap, AOT.Exp, bias=bias_sb[:])
                nc.scalar.activation(dst[:], e[:], AOT.Ln, bias=1.0)

            c0_sb = wpool.tile([8, NPIX], F32, tag="f32buf2")
            nc.sync.dma_start(out=c0_sb[:], in_=cc0_out[:])
            h1_f32 = wpool.tile([8, NPIX], F32, tag="hbuf")
            softplus_to(h1_f32, c0_sb[:], b0_sb, 8, "sp1")

            def ln_small(h_f32, nparts, n_elems, tag):
                hsum = wpool.tile([nparts, 1], F32, tag=tag + "_hsum")
                nc.vector.tensor_reduce(hsum[:], h_f32[:], mybir.AxisListType.X,
                                        ALU.add)
                hsq = wpool.tile([nparts, NPIX], BF16, tag="sq_scratch2")
                hsumsq = wpool.tile([nparts, 1], F32, tag=tag + "_hsumsq")
                nc.scalar.activation(hsq[:], h_f32[:], AOT.Square,
                                     accum_out=hsumsq[:])
                st2 = wpool.tile([nparts, 2], F32, tag=tag + "_st2")
                nc.vector.tensor_copy(st2[:, 0:1], hsum[:])
                nc.vector.tensor_copy(st2[:, 1:2], hsumsq[:])
                st_ps = psB.tile([128, 2], F32, tag="acc", bufs=2)
                nc.tensor.matmul(st_ps[:], lhsT=onesbc_sb[0:nparts, :],
                                 rhs=st2[:], start=True, stop=True)
                st_bc = wpool.tile([nparts, 2], F32, tag=tag + "_stbc")
                nc.scalar.activation(st_bc[:], st_ps[0:nparts, :], AOT.Copy)
                return ln_from_bc(st_bc, n_elems, nparts, tag)

            mu1_bc, istd1_bc = ln_small(h1_f32, 8, LN1_N, "ln1")
            xn1 = wpool.tile([8, NPIX], BF16, tag="xn1")
            nc.vector.tensor_scalar(xn1[:], h1_f32[:], mu1_bc[:], istd1_bc[:],
                                    ALU.subtract, ALU.mult)

            # ---------- Phase B8: layers 1 and 2 (replicated) ----------
            def spade_small(xn_bf, nco, pss, opg_sb, bbea_sb, tag):
                """pss: prefired conv PSUMs [32+nco, 512] per half; gamma at
                partitions 0:nco, beta at 32:32+nco."""
                sp_ = wpool.tile([nco, NPIX], BF16, tag=f"{tag}_sp")
                for h in range(2):
                    p = pss[h]
                    hs = slice(h * 512, (h + 1) * 512)
                    u_ = wpool.tile([nco, 512], BF16, tag=f"{tag}_u")
                    nc.vector.scalar_tensor_tensor(u_[:], p[0:nco, :],
                                                   opg_sb[:], xn_bf[:, hs],
                                                   ALU.add, ALU.mult)
                    nc.vector.scalar_tensor_tensor(sp_[:, hs], p[32:32 + nco, :],
                                                   bbea_sb[:], u_[:],
                                                   ALU.add, ALU.add)
                return sp_

            sp1 = spade_small(xn1, 8, l1_ps, opg1_sb, bbe1a_sb, "l1")
            c1_sb = wpool.tile([16, NPIX], F32, tag="f32buf")
            for h in range(2):
                c1_ps = psB.tile([16, 512], F32, tag="acc", bufs=2)
                nc.tensor.matmul(c1_ps[:], lhsT=wc1t_sb[:],
                                 rhs=sp1[:, h * 512:(h + 1) * 512],
                                 start=True, stop=True)
                nc.scalar.activation(c1_sb[:, h * 512:(h + 1) * 512], c1_ps[:],
                                     AOT.Copy)
            h2_f32 = wpool.tile([16, NPIX], F32, tag="hbuf2")
            softplus_to(h2_f32, c1_sb[:], b1_sb, 16, "sp2")

            mu2_bc, istd2_bc = ln_small(h2_f32, 16, LN2_N, "ln2")
            xn2 = wpool.tile([16, NPIX], BF16, tag="xn2")
            nc.vector.tensor_scalar(xn2[:], h2_f32[:], mu2_bc[:], istd2_bc[:],
                                    ALU.subtract, ALU.mult)

            sp2 = spade_small(xn2, 16, l2_ps, opg2_sb, bbe2a_sb, "l2")
            c2_sb = wpool.tile([1, NPIX], F32, tag="f32buf3")
            for h in range(2):
                c2_ps = psB.tile([1, 512], F32, tag="acc", bufs=2)
                nc.tensor.matmul(c2_ps[:], lhsT=wc2t_sb[:],
                                 rhs=sp2[:, h * 512:(h + 1) * 512],
                                 start=True, stop=True)
                nc.scalar.activation(c2_sb[:, h * 512:(h + 1) * 512], c2_ps[:],
                                     AOT.Copy)
            out_f32 = wpool.tile([1, NPIX], F32, tag="hbuf3")
            softplus_to(out_f32, c2_sb[:], b2_sb, 1, "sp3")
            nc.sync.dma_start(out=out_t[:], in_=out_f32[:])
            psB_ctx.__exit__(None, None, None)

    nc.finalize()
    _split_sync_waits(nc)
    return nc


def _pack_inputs(inputs):
    f32 = np.float32
    R = _resize_matrix(HI, HP)      # [32, 448]
    C = _resize_matrix(WI, WP)      # [32, 448]
    rt = np.zeros((112, 4, 32), f32)
    ctm = np.zeros((112, 4, 32), f32)
    for c in range(4):
        rt[:, c, :] = R[:, c * 112:(c + 1) * 112].T
        ctm[:, c, :] = C[:, c * 112:(c + 1) * 112].T
    ident = np.eye(32, dtype=f32)

    segmap = inputs["segmap"]            # [2, 448, 448] int32
    f_sem = inputs["f_semantic"]         # [2, 384, 32, 32]
    x_main = inputs["x_main"]            # [2, 1536, 32, 32]
    rows = (np.arange(HP) * HI) // HP

    def tap_t(w):  # [co, ci, 3, 3] -> [ci, 9, co]
        return np.ascontiguousarray(w.transpose(1, 2, 3, 0).reshape(
            w.shape[1], 9, w.shape[0]))

    # shared conv layers packed as [ci_p, tap, cic, co]
    def sh_pack(L):
        return tap_t(inputs[f"w_sh{L}"]).reshape(3, 128, 9, HM).transpose(
            1, 2, 0, 3).astype(BF16_NP)
    wsh_pk = [sh_pack(L) for L in range(3)]
    bsh_pk = [inputs[f"b_sh{L}"].reshape(128, 1).astype(f32) for L in range(3)]

    maps = []
    for cid in range(8):
        b, g = cid // G, cid % G
        d = {}
        seg = segmap[b].astype(f32)
        d["segbf"] = seg.reshape(4, 112, WI).transpose(1, 0, 2).astype(BF16_NP)
        d["segval"] = np.broadcast_to(
            (np.arange(SEGC, dtype=f32) + SEGC * g)[None, :], (128, SEGC)
        ).astype(f32).copy()
        d["rt"] = rt.astype(BF16_NP)
        d["ct"] = ctm.astype(BF16_NP)
        d["ident"] = ident.astype(BF16_NP)
        seg_small = seg[rows[:, None], rows[None, :]].reshape(-1)   # [1024]
        d["segsm"] = seg_small.reshape(8, 128).T.astype(BF16_NP).copy()
        fT = f_sem[b].reshape(CS, NPIX).T                           # [1024, 384]
        fTe = np.concatenate([fT, np.ones((NPIX, 1), f32)], 1)      # [1024, 385]
        d["fsemT"] = fTe.reshape(8, 128, 385).transpose(1, 0, 2).astype(
            BF16_NP).copy()
        xc = x_main[b, g * COC:(g + 1) * COC].reshape(COC, NPIX)
        d["xq"] = np.ascontiguousarray(
            xc.reshape(3, 128, NPIX).transpose(1, 0, 2))
        d["wsh0"] = wsh_pk[0]
        d["bsh0"] = bsh_pk[0]
        mine = 1 if g < 2 else 2
        d["wshm"] = wsh_pk[mine]
        d["bshm"] = bsh_pk[mine]
        cosl = slice(g * COC, (g + 1) * COC)
        d["wg"] = tap_t(inputs["w_g0"][cosl]).astype(BF16_NP)      # [128,9,384]
        d["wbe"] = tap_t(inputs["w_be0"][cosl]).astype(BF16_NP)
        d["opg0"] = np.ascontiguousarray(
            (1.0 + inputs["b_g0"][cosl]).reshape(3, 128).T).astype(f32)
        d["bbe0a"] = np.ascontiguousarray(
            inputs["b_be0"][cosl].reshape(3, 128).T).astype(f32)
        wc0 = inputs["w_c0"][:, :, 0, 0]                           # [8, 1536]
        d["wc0t"] = np.ascontiguousarray(
            wc0[:, cosl].T.reshape(3, 128, 8).transpose(1, 0, 2)).astype(BF16_NP)
        def gbe_pack(wg_, wbe_, nco):   # [128, 9, 32+nco], beta at col 32
            out = np.zeros((128, 9, 32 + nco), f32)
            out[:, :, 0:nco] = tap_t(wg_)
            out[:, :, 32:32 + nco] = tap_t(wbe_)
            return out.astype(BF16_NP)

        d["wgbe1"] = gbe_pack(inputs["w_g1"], inputs["w_be1"], 8)
        d["opg1"] = (1.0 + inputs["b_g1"]).reshape(8, 1).astype(f32)
        d["bbe1a"] = inputs["b_be1"].reshape(8, 1).astype(f32)
        d["wgbe2"] = gbe_pack(inputs["w_g2"], inputs["w_be2"], 16)
        d["opg2"] = (1.0 + inputs["b_g2"]).reshape(16, 1).astype(f32)
        d["bbe2a"] = inputs["b_be2"].reshape(16, 1).astype(f32)
        d["wc1t"] = inputs["w_c1"][:, :, 0, 0].T.astype(BF16_NP).copy()  # [8,16]
        d["wc2t"] = inputs["w_c2"][:, :, 0, 0].T.astype(BF16_NP).copy()  # [16,1]
        d["b0"] = inputs["bias0"].reshape(8, 1).astype(f32)
        d["b1"] = inputs["bias1"].reshape(16, 1).astype(f32)
        d["b2"] = inputs["bias2"].reshape(1, 1).astype(f32)
        d["ones128f"] = np.ones((128, 1), f32)
        d["onesbc"] = np.ones((16, 128), f32)
        maps.append(d)
    return maps


def kernel(**inputs):
    if "nc" not in _NC_CACHE:
        _NC_CACHE["nc"] = _build_nc()
    nc = _NC_CACHE["nc"]
    in_maps = _pack_inputs(inputs)
    res = run_bass_kernel_spmd(nc, in_maps, list(range(8)))
    out = np.zeros((B, 1, HP, WP), np.float32)
    out[0, 0] = res.results[0]["out"].reshape(HP, WP)
    out[1, 0] = res.results[4]["out"].reshape(HP, WP)
    return out


if __name__ == "__main__":
    nc = _build_nc()
    print("built OK; instructions:",
          sum(len(b.instructions) for f in nc.m.functions for b in f.blocks))
